# revision 4
# baseline (speedup 1.0000x reference)
"""Trainium2 Bass kernel for nn_BaselineModel_35175782154746 (dense transformer
block with SiLU attention + relative-position bias).

Sharding: 8 NeuronCores = 4 batches x 2 head-groups (8 heads each).
Each core computes, for its (batch b, head-group g):
    U, Q, K, V projections (columns g*1024:(g+1)*1024 of Wu/Wq/Wk/Wv),
    SiLU attention with rel-pos bias for its 8 heads,
    gated = out * U, partial = gated @ Wf2[g*1024:(g+1)*1024, :].
Host reduces: out[b] = partial[2b] + partial[2b+1] + bf2.

All matmuls run as float32r (full fp32 storage, fast PE mode) with N=512
moving dim. Activations/bias layouts are arranged so the contraction dim is
always on SBUF partitions (inputs are pre-transposed on host).
"""

import sys
import os

for _p in ("/root/.axon_site/_ro/trn_rl_repo", "/opt/trn_rl_repo"):
    if os.path.isdir(_p) and _p not in sys.path:
        sys.path.append(_p)

import numpy as np

import concourse.bass as bass
import concourse.mybir as mybir
import concourse.tile as tile
from concourse import bacc
from concourse.bass_utils import run_bass_kernel_spmd

B, S, H, NH, MAXLEN = 4, 1024, 2048, 16, 1024
HD = H // NH            # 128
NHL = 8                 # heads per core (local)
HGRP = 2                # head groups
NCORES = 8
KT16 = H // 128         # 16 k-tiles for the H contraction
SCALE = float(HD) ** -0.5

f32 = mybir.dt.float32
f32r = mybir.dt.float32r
bf16 = mybir.dt.bfloat16
SILU = mybir.ActivationFunctionType.Silu
MULT = mybir.AluOpType.mult
ADD = mybir.AluOpType.add

TRACE = False
LAST_EXEC_NS = None
_CACHE = {}


def _build(causal: bool):
    nc = bacc.Bacc("TRN2", target_bir_lowering=False, debug=False,
                   num_devices=NCORES)

    def din(name, shape, dt=f32):
        return nc.dram_tensor(name, shape, dt, kind="ExternalInput").ap()

    qT = din("qT", [H, S], f32r)
    kT = din("kT", [H, S], f32r)
    vT = din("vT", [H, S], f32r)
    wq = din("wq", [H, NHL * HD], f32r)
    wk = din("wk", [H, NHL * HD], f32r)
    wv = din("wv", [H, NHL * HD], f32r)
    wu = din("wu", [H, NHL * HD], f32r)
    wf2 = din("wf2", [NHL * HD, H], f32r)
    bq = din("bq", [128, NHL])
    bk = din("bk", [128, NHL])
    bu = din("bu", [128, NHL])
    bv = din("bv", [128, NHL * HD])
    atab = din("atab", [NHL, 128, 2047])
    if causal:
        maskd = din("maskd", [128, NHL, 512])
    else:
        maskf = din("maskf", [128, NHL, S])
    out = nc.dram_tensor("out", [S, H], f32, kind="ExternalOutput").ap()

    with tile.TileContext(nc) as tc:
        with (
            tc.tile_pool(name="const", bufs=1) as constp,
            tc.tile_pool(name="gatedp", bufs=1) as gatedp,
        ):
            bq_t = constp.tile([128, NHL], f32, tag="bq")
            bk_t = constp.tile([128, NHL], f32, tag="bk")
            bu_t = constp.tile([128, NHL], f32, tag="bu")
            bv_t = constp.tile([128, NHL * HD], f32, tag="bv")
            nc.sync.dma_start(bq_t[:], bq[:])
            nc.sync.dma_start(bk_t[:], bk[:])
            nc.sync.dma_start(bu_t[:], bu[:])
            nc.sync.dma_start(bv_t[:], bv[:])

            gatedT = gatedp.tile([128, NHL, S], f32r, tag="gatedT")

            with tc.tile_pool(name="attres", bufs=1) as attres:
                UT = attres.tile([128, NHL, S], bf16, tag="UT")
                QT = attres.tile([128, NHL, S], f32r, tag="QT")
                KTt = attres.tile([128, NHL, S], f32r, tag="KT")
                V = attres.tile([128, NHL, S], f32r, tag="V")
                if causal:
                    mask_t = attres.tile([128, NHL, 512], f32, tag="mask")
                    nc.sync.dma_start(mask_t[:], maskd[:])
                else:
                    mask_t = attres.tile([128, NHL, S], f32, tag="mask")
                    nc.sync.dma_start(mask_t[:], maskf[:])

                # ---- projections U, Q, K (transposed outputs [HD, S]) ----
                with (
                    tc.tile_pool(name="xin", bufs=3) as xinp,
                    tc.tile_pool(name="win", bufs=3) as winp,
                    tc.tile_pool(name="pps", bufs=1, space="PSUM") as ppsum,
                ):
                    for wdram, indram, btile, outtile in (
                        (wu, qT, bu_t, UT),
                        (wq, qT, bq_t, QT),
                        (wk, kT, bk_t, KTt),
                    ):
                        for ih in range(2):
                            ps = [ppsum.tile([128, 512], f32, tag=f"pp{h}", name=f"pp{h}")
                                  for h in range(NHL)]
                            for k in range(KT16):
                                xt = xinp.tile([128, 512], f32r, tag="xin")
                                nc.sync.dma_start(
                                    xt[:],
                                    indram[k * 128:(k + 1) * 128,
                                           ih * 512:(ih + 1) * 512])
                                wt = winp.tile([128, NHL * HD], f32r, tag="win")
                                nc.sync.dma_start(
                                    wt[:], wdram[k * 128:(k + 1) * 128, :])
                                for h in range(NHL):
                                    nc.tensor.matmul(
                                        ps[h][:],
                                        lhsT=wt[:, h * HD:(h + 1) * HD],
                                        rhs=xt[:],
                                        start=(k == 0), stop=(k == KT16 - 1))
                            for h in range(NHL):
                                nc.scalar.activation(
                                    outtile[:, h, ih * 512:(ih + 1) * 512],
                                    ps[h][:], SILU, bias=btile[:, h:h + 1])

                    # ---- projection V (natural layout [S, NHL*HD]) ----
                    for ch in range(2):
                        ps = [ppsum.tile([128, 512], f32, tag=f"pp{sb}", name=f"ppv{sb}")
                              for sb in range(8)]
                        for k in range(KT16):
                            vt = xinp.tile([128, S], f32r, tag="vin")
                            nc.sync.dma_start(
                                vt[:], vT[k * 128:(k + 1) * 128, :])
                            wt = winp.tile([128, 512], f32r, tag="wvin")
                            nc.sync.dma_start(
                                wt[:], wv[k * 128:(k + 1) * 128,
                                          ch * 512:(ch + 1) * 512])
                            for sb in range(8):
                                nc.tensor.matmul(
                                    ps[sb][:],
                                    lhsT=vt[:, sb * 128:(sb + 1) * 128],
                                    rhs=wt[:],
                                    start=(k == 0), stop=(k == KT16 - 1))
                        for sb in range(8):
                            nc.vector.tensor_add(
                                V[:, sb, ch * 512:(ch + 1) * 512],
                                ps[sb][:],
                                bv_t[:, ch * 512:(ch + 1) * 512])
                            nc.scalar.activation(
                                V[:, sb, ch * 512:(ch + 1) * 512],
                                V[:, sb, ch * 512:(ch + 1) * 512], SILU)

                # ---- attention per head ----
                with (
                    tc.tile_pool(name="atabp", bufs=2) as atabp,
                    tc.tile_pool(name="attnp", bufs=4) as attnp,
                    tc.tile_pool(name="pssc", bufs=3, space="PSUM") as pssc,
                    tc.tile_pool(name="psav", bufs=2, space="PSUM") as psav,
                ):
                    for h in range(NHL):
                        at = atabp.tile([128, 2047], f32, tag="atab")
                        nc.sync.dma_start(at[:], atab[h])
                        for ih in range(2):
                            njb = (4 * ih + 4) if causal else 8
                            avp = psav.tile([128, 512], f32, tag="av")
                            for jb in range(njb):
                                scp = pssc.tile([128, 512], f32, tag="sc")
                                nc.tensor.matmul(
                                    scp[:],
                                    lhsT=KTt[:, h, jb * 128:(jb + 1) * 128],
                                    rhs=QT[:, h, ih * 512:(ih + 1) * 512],
                                    start=True, stop=True)
                                att = attnp.tile([128, 512], f32r, tag="attn")
                                d0 = ih * 512 - jb * 128 + MAXLEN - 1
                                nc.vector.scalar_tensor_tensor(
                                    att[:], scp[:], SCALE, at[:, d0:d0 + 512],
                                    op0=MULT, op1=ADD)
                                nc.scalar.activation(att[:], att[:], SILU)
                                if causal:
                                    if jb // 4 == ih:  # diagonal-straddling tile
                                        nc.vector.tensor_mul(
                                            att[:], att[:], mask_t[:, jb, :])
                                else:
                                    nc.vector.tensor_mul(
                                        att[:], att[:],
                                        mask_t[:, jb, ih * 512:(ih + 1) * 512])
                                nc.tensor.matmul(
                                    avp[:],
                                    lhsT=V[:, jb, h * HD:(h + 1) * HD],
                                    rhs=att[:],
                                    start=(jb == 0), stop=(jb == njb - 1))
                            nc.vector.tensor_mul(
                                gatedT[:, h, ih * 512:(ih + 1) * 512],
                                avp[:],
                                UT[:, h, ih * 512:(ih + 1) * 512])

            # ---- f2: partial = gatedT.T @ Wf2 (bf2 added on host) ----
            with (
                tc.tile_pool(name="wf2p", bufs=2) as wf2p,
                tc.tile_pool(name="stgp", bufs=3) as stgp,
                tc.tile_pool(name="psf2", bufs=4, space="PSUM") as psf2,
            ):
                wf2r = wf2.rearrange("(cb p) n -> p cb n", p=128)
                for n in range(4):
                    w2 = wf2p.tile([128, NHL, 512], f32r, tag="wf2")
                    nc.sync.dma_start(w2[:], wf2r[:, :, n * 512:(n + 1) * 512])
                    for sb in range(8):
                        ps = psf2.tile([128, 512], f32, tag="f2")
                        for cb in range(NHL):
                            nc.tensor.matmul(
                                ps[:],
                                lhsT=gatedT[:, cb, sb * 128:(sb + 1) * 128],
                                rhs=w2[:, cb, :],
                                start=(cb == 0), stop=(cb == NHL - 1))
                        st = stgp.tile([128, 512], f32, tag="st")
                        nc.vector.tensor_copy(st[:], ps[:])
                        nc.sync.dma_start(
                            out[sb * 128:(sb + 1) * 128,
                                n * 512:(n + 1) * 512], st[:])

    nc.compile()
    return nc


def _host_shards(query, key, value, attn_mask, Wq, bq, Wk, bk, Wv, bv,
                 Wu, bu, Wf2, rel_table, causal):
    """Build the per-core input maps."""
    in_maps = []
    # precompute per-head-group weight slices once (shared by 4 cores each)
    gdata = []
    for g in range(HGRP):
        c0, c1 = g * NHL * HD, (g + 1) * NHL * HD
        wq_c = np.ascontiguousarray(Wq[:, c0:c1])
        wk_c = np.ascontiguousarray(Wk[:, c0:c1])
        wv_c = np.ascontiguousarray(Wv[:, c0:c1])
        wu_c = np.ascontiguousarray(Wu[:, c0:c1])
        wf2_c = np.ascontiguousarray(Wf2[c0:c1, :])
        bq_c = np.ascontiguousarray(bq[c0:c1].reshape(NHL, 128).T)
        bk_c = np.ascontiguousarray(bk[c0:c1].reshape(NHL, 128).T)
        bu_c = np.ascontiguousarray(bu[c0:c1].reshape(NHL, 128).T)
        bv_c = np.ascontiguousarray(
            np.broadcast_to(bv[c0:c1][None, :], (128, NHL * HD)))
        # atab[h, r, y] = rel_table[y - r, g*NHL + h]  (0 where y < r)
        y = np.arange(2047)[None, :]
        r = np.arange(128)[:, None]
        idx = y - r                      # [128, 2047]
        valid = (idx >= 0) & (idx <= 2 * MAXLEN - 2)
        idxc = np.clip(idx, 0, 2 * MAXLEN - 2)
        cols = rel_table[:, g * NHL:(g + 1) * NHL]   # [2047, NHL]
        at = cols[idxc] * valid[:, :, None]          # [128, 2047, NHL]
        atab_c = np.ascontiguousarray(
            at.transpose(2, 0, 1)).astype(np.float32)  # [NHL, 128, 2047]
        gdata.append((wq_c, wk_c, wv_c, wu_c, wf2_c, bq_c, bk_c, bu_c,
                      bv_c, atab_c))

    for c in range(NCORES):
        b, g = c // HGRP, c % HGRP
        (wq_c, wk_c, wv_c, wu_c, wf2_c, bq_c, bk_c, bu_c, bv_c,
         atab_c) = gdata[g]
        m = {
            "qT": np.ascontiguousarray(query[b].T),
            "kT": np.ascontiguousarray(key[b].T),
            "vT": np.ascontiguousarray(value[b].T),
            "wq": wq_c, "wk": wk_c, "wv": wv_c, "wu": wu_c, "wf2": wf2_c,
            "bq": bq_c, "bk": bk_c, "bu": bu_c, "bv": bv_c, "atab": atab_c,
        }
        mb = attn_mask[b]
        if causal:
            # maskd[r, jb, c] = mask[i0(jb)+c, jb*128+r], i0 = (jb//4)*512
            md = np.empty((128, NHL, 512), np.float32)
            for jb in range(8):
                i0 = (jb // 4) * 512
                md[:, jb, :] = mb[i0:i0 + 512, jb * 128:(jb + 1) * 128].T
            m["maskd"] = md
        else:
            mf = np.empty((128, NHL, S), np.float32)
            for jb in range(8):
                mf[:, jb, :] = mb[:, jb * 128:(jb + 1) * 128].T
            m["maskf"] = mf
        in_maps.append(m)
    return in_maps


def kernel(query, key, value, attn_mask, Wq, bq, Wk, bk, Wv, bv, Wu, bu,
           Wf2, bf2, rel_table):
    global LAST_EXEC_NS
    query = np.asarray(query, np.float32)
    key = np.asarray(key, np.float32)
    value = np.asarray(value, np.float32)
    attn_mask = np.asarray(attn_mask, bool)
    Wq, bq = np.asarray(Wq, np.float32), np.asarray(bq, np.float32)
    Wk, bk = np.asarray(Wk, np.float32), np.asarray(bk, np.float32)
    Wv, bv = np.asarray(Wv, np.float32), np.asarray(bv, np.float32)
    Wu, bu = np.asarray(Wu, np.float32), np.asarray(bu, np.float32)
    Wf2, bf2 = np.asarray(Wf2, np.float32), np.asarray(bf2, np.float32)
    rel_table = np.asarray(rel_table, np.float32)

    tril = np.tril(np.ones((S, S), bool))
    causal = all(np.array_equal(attn_mask[b], tril) for b in range(B))

    if causal not in _CACHE:
        _CACHE[causal] = _build(causal)
    nc = _CACHE[causal]

    in_maps = _host_shards(query, key, value, attn_mask, Wq, bq, Wk, bk,
                           Wv, bv, Wu, bu, Wf2, rel_table, causal)

    res = run_bass_kernel_spmd(nc, in_maps, list(range(NCORES)), trace=TRACE)
    if res.exec_time_ns is not None:
        LAST_EXEC_NS = res.exec_time_ns

    outp = np.empty((B, S, H), np.float32)
    for b in range(B):
        outp[b] = (res.results[2 * b]["out"] + res.results[2 * b + 1]["out"]
                   + bf2[None, :])
    return outp


# revision 5
# speedup vs baseline: 1.2066x; 1.2066x over previous
"""Trainium2 Bass kernel for nn_BaselineModel_35175782154746 (dense transformer
block with SiLU attention + relative-position bias).

Sharding: 8 NeuronCores = 4 batches x 2 head-groups (8 heads each).
Each core computes, for its (batch b, head-group g):
    U, Q, K, V projections (columns g*1024:(g+1)*1024 of Wu/Wq/Wk/Wv),
    SiLU attention with rel-pos bias for its 8 heads,
    gated = out * U, partial = gated @ Wf2[g*1024:(g+1)*1024, :].
Host reduces: out[b] = partial[2b] + partial[2b+1] + bf2.

All matmuls run as float32r (full fp32 storage, fast PE mode) with N=512
moving dim. Activations/bias layouts are arranged so the contraction dim is
always on SBUF partitions (inputs are pre-transposed on host).
"""

import sys
import os

for _p in ("/root/.axon_site/_ro/trn_rl_repo", "/opt/trn_rl_repo"):
    if os.path.isdir(_p) and _p not in sys.path:
        sys.path.append(_p)

import numpy as np

import concourse.bass as bass
import concourse.mybir as mybir
import concourse.tile as tile
from concourse import bacc
from concourse.bass_utils import run_bass_kernel_spmd

B, S, H, NH, MAXLEN = 4, 1024, 2048, 16, 1024
HD = H // NH            # 128
NHL = 8                 # heads per core (local)
HGRP = 2                # head groups
NCORES = 8
KT16 = H // 128         # 16 k-tiles for the H contraction
SCALE = float(HD) ** -0.5

f32 = mybir.dt.float32
f32r = mybir.dt.float32r
bf16 = mybir.dt.bfloat16
SILU = mybir.ActivationFunctionType.Silu
MULT = mybir.AluOpType.mult
ADD = mybir.AluOpType.add

TRACE = False
LAST_EXEC_NS = None
MM_DT = "bf16"          # "bf16" or "f32r" matmul operand dtype
_CACHE = {}


def _build(causal: bool, mm_dt=None):
    mmdt = {"bf16": bf16, "f32r": f32r}[mm_dt or MM_DT]
    nc = bacc.Bacc("TRN2", target_bir_lowering=False, debug=False,
                   num_devices=NCORES)

    def din(name, shape, dt=f32):
        return nc.dram_tensor(name, shape, dt, kind="ExternalInput").ap()

    qT = din("qT", [H, S], mmdt)
    kT = din("kT", [H, S], mmdt)
    vT = din("vT", [H, S], mmdt)
    wq = din("wq", [H, NHL * HD], mmdt)
    wk = din("wk", [H, NHL * HD], mmdt)
    wv = din("wv", [H, NHL * HD], mmdt)
    wu = din("wu", [H, NHL * HD], mmdt)
    wf2 = din("wf2", [NHL * HD, H], mmdt)
    bq = din("bq", [128, NHL])
    bk = din("bk", [128, NHL])
    bu = din("bu", [128, NHL])
    bv = din("bv", [128, NHL * HD])
    atab = din("atab", [NHL, 128, 2047])
    if causal:
        maskd = din("maskd", [128, NHL, 512])
    else:
        maskf = din("maskf", [128, NHL, S])
    out = nc.dram_tensor("out", [S, H], f32, kind="ExternalOutput").ap()

    with tile.TileContext(nc) as tc:
        with (
            tc.tile_pool(name="const", bufs=1) as constp,
            tc.tile_pool(name="gatedp", bufs=1) as gatedp,
        ):
            bq_t = constp.tile([128, NHL], f32, tag="bq")
            bk_t = constp.tile([128, NHL], f32, tag="bk")
            bu_t = constp.tile([128, NHL], f32, tag="bu")
            bv_t = constp.tile([128, NHL * HD], f32, tag="bv")
            nc.sync.dma_start(bq_t[:], bq[:])
            nc.sync.dma_start(bk_t[:], bk[:])
            nc.sync.dma_start(bu_t[:], bu[:])
            nc.sync.dma_start(bv_t[:], bv[:])

            gatedT = gatedp.tile([128, NHL, S], mmdt, tag="gatedT")

            with tc.tile_pool(name="attres", bufs=1) as attres:
                UT = attres.tile([128, NHL, S], bf16, tag="UT")
                QT = attres.tile([128, NHL, S], mmdt, tag="QT")
                KTt = attres.tile([128, NHL, S], mmdt, tag="KT")
                V = attres.tile([128, NHL, S], mmdt, tag="V")
                if causal:
                    mask_t = attres.tile([128, NHL, 512], f32, tag="mask")
                    nc.sync.dma_start(mask_t[:], maskd[:])
                else:
                    mask_t = attres.tile([128, NHL, S], f32, tag="mask")
                    nc.sync.dma_start(mask_t[:], maskf[:])

                # ---- projections U, Q, K (transposed outputs [HD, S]) ----
                with (
                    tc.tile_pool(name="xin", bufs=3) as xinp,
                    tc.tile_pool(name="win", bufs=3) as winp,
                    tc.tile_pool(name="pps", bufs=1, space="PSUM") as ppsum,
                ):
                    for wdram, indram, btile, outtile in (
                        (wu, qT, bu_t, UT),
                        (wq, qT, bq_t, QT),
                        (wk, kT, bk_t, KTt),
                    ):
                        for ih in range(2):
                            ps = [ppsum.tile([128, 512], f32, tag=f"pp{h}", name=f"pp{h}")
                                  for h in range(NHL)]
                            for k in range(KT16):
                                xt = xinp.tile([128, 512], mmdt, tag="xin")
                                nc.sync.dma_start(
                                    xt[:],
                                    indram[k * 128:(k + 1) * 128,
                                           ih * 512:(ih + 1) * 512])
                                wt = winp.tile([128, NHL * HD], mmdt, tag="win")
                                nc.sync.dma_start(
                                    wt[:], wdram[k * 128:(k + 1) * 128, :])
                                for h in range(NHL):
                                    nc.tensor.matmul(
                                        ps[h][:],
                                        lhsT=wt[:, h * HD:(h + 1) * HD],
                                        rhs=xt[:],
                                        start=(k == 0), stop=(k == KT16 - 1))
                            for h in range(NHL):
                                nc.scalar.activation(
                                    outtile[:, h, ih * 512:(ih + 1) * 512],
                                    ps[h][:], SILU, bias=btile[:, h:h + 1])

                    # ---- projection V (natural layout [S, NHL*HD]) ----
                    for ch in range(2):
                        ps = [ppsum.tile([128, 512], f32, tag=f"pp{sb}", name=f"ppv{sb}")
                              for sb in range(8)]
                        for k in range(KT16):
                            vt = xinp.tile([128, S], mmdt, tag="vin")
                            nc.sync.dma_start(
                                vt[:], vT[k * 128:(k + 1) * 128, :])
                            wt = winp.tile([128, 512], mmdt, tag="wvin")
                            nc.sync.dma_start(
                                wt[:], wv[k * 128:(k + 1) * 128,
                                          ch * 512:(ch + 1) * 512])
                            for sb in range(8):
                                nc.tensor.matmul(
                                    ps[sb][:],
                                    lhsT=vt[:, sb * 128:(sb + 1) * 128],
                                    rhs=wt[:],
                                    start=(k == 0), stop=(k == KT16 - 1))
                        for sb in range(8):
                            nc.vector.tensor_add(
                                V[:, sb, ch * 512:(ch + 1) * 512],
                                ps[sb][:],
                                bv_t[:, ch * 512:(ch + 1) * 512])
                            nc.scalar.activation(
                                V[:, sb, ch * 512:(ch + 1) * 512],
                                V[:, sb, ch * 512:(ch + 1) * 512], SILU)

                # ---- attention per head ----
                with (
                    tc.tile_pool(name="atabp", bufs=2) as atabp,
                    tc.tile_pool(name="attnp", bufs=4) as attnp,
                    tc.tile_pool(name="pssc", bufs=3, space="PSUM") as pssc,
                    tc.tile_pool(name="psav", bufs=2, space="PSUM") as psav,
                ):
                    for h in range(NHL):
                        at = atabp.tile([128, 2047], f32, tag="atab")
                        nc.sync.dma_start(at[:], atab[h])
                        for ih in range(2):
                            njb = (4 * ih + 4) if causal else 8
                            avp = psav.tile([128, 512], f32, tag="av")
                            for jb in range(njb):
                                scp = pssc.tile([128, 512], f32, tag="sc")
                                nc.tensor.matmul(
                                    scp[:],
                                    lhsT=KTt[:, h, jb * 128:(jb + 1) * 128],
                                    rhs=QT[:, h, ih * 512:(ih + 1) * 512],
                                    start=True, stop=True)
                                att = attnp.tile([128, 512], mmdt, tag="attn")
                                d0 = ih * 512 - jb * 128 + MAXLEN - 1
                                nc.vector.scalar_tensor_tensor(
                                    att[:], scp[:], SCALE, at[:, d0:d0 + 512],
                                    op0=MULT, op1=ADD)
                                nc.scalar.activation(att[:], att[:], SILU)
                                if causal:
                                    if jb // 4 == ih:  # diagonal-straddling tile
                                        nc.vector.tensor_mul(
                                            att[:], att[:], mask_t[:, jb, :])
                                else:
                                    nc.vector.tensor_mul(
                                        att[:], att[:],
                                        mask_t[:, jb, ih * 512:(ih + 1) * 512])
                                nc.tensor.matmul(
                                    avp[:],
                                    lhsT=V[:, jb, h * HD:(h + 1) * HD],
                                    rhs=att[:],
                                    start=(jb == 0), stop=(jb == njb - 1))
                            nc.vector.tensor_mul(
                                gatedT[:, h, ih * 512:(ih + 1) * 512],
                                avp[:],
                                UT[:, h, ih * 512:(ih + 1) * 512])

            # ---- f2: partial = gatedT.T @ Wf2 (bf2 added on host) ----
            with (
                tc.tile_pool(name="wf2p", bufs=2) as wf2p,
                tc.tile_pool(name="stgp", bufs=3) as stgp,
                tc.tile_pool(name="psf2", bufs=4, space="PSUM") as psf2,
            ):
                wf2r = wf2.rearrange("(cb p) n -> p cb n", p=128)
                for n in range(4):
                    w2 = wf2p.tile([128, NHL, 512], mmdt, tag="wf2")
                    nc.sync.dma_start(w2[:], wf2r[:, :, n * 512:(n + 1) * 512])
                    for sb in range(8):
                        ps = psf2.tile([128, 512], f32, tag="f2")
                        for cb in range(NHL):
                            nc.tensor.matmul(
                                ps[:],
                                lhsT=gatedT[:, cb, sb * 128:(sb + 1) * 128],
                                rhs=w2[:, cb, :],
                                start=(cb == 0), stop=(cb == NHL - 1))
                        st = stgp.tile([128, 512], f32, tag="st")
                        nc.vector.tensor_copy(st[:], ps[:])
                        nc.sync.dma_start(
                            out[sb * 128:(sb + 1) * 128,
                                n * 512:(n + 1) * 512], st[:])

    nc.compile()
    return nc


def _host_shards(query, key, value, attn_mask, Wq, bq, Wk, bk, Wv, bv,
                 Wu, bu, Wf2, rel_table, causal, mm_dt=None):
    """Build the per-core input maps."""
    import ml_dtypes
    npdt = (np.dtype(ml_dtypes.bfloat16) if (mm_dt or MM_DT) == "bf16"
            else np.float32)
    in_maps = []
    # precompute per-head-group weight slices once (shared by 4 cores each)
    gdata = []
    for g in range(HGRP):
        c0, c1 = g * NHL * HD, (g + 1) * NHL * HD
        wq_c = np.ascontiguousarray(Wq[:, c0:c1]).astype(npdt)
        wk_c = np.ascontiguousarray(Wk[:, c0:c1]).astype(npdt)
        wv_c = np.ascontiguousarray(Wv[:, c0:c1]).astype(npdt)
        wu_c = np.ascontiguousarray(Wu[:, c0:c1]).astype(npdt)
        wf2_c = np.ascontiguousarray(Wf2[c0:c1, :]).astype(npdt)
        bq_c = np.ascontiguousarray(bq[c0:c1].reshape(NHL, 128).T)
        bk_c = np.ascontiguousarray(bk[c0:c1].reshape(NHL, 128).T)
        bu_c = np.ascontiguousarray(bu[c0:c1].reshape(NHL, 128).T)
        bv_c = np.ascontiguousarray(
            np.broadcast_to(bv[c0:c1][None, :], (128, NHL * HD)))
        # atab[h, r, y] = rel_table[y - r, g*NHL + h]  (0 where y < r)
        y = np.arange(2047)[None, :]
        r = np.arange(128)[:, None]
        idx = y - r                      # [128, 2047]
        valid = (idx >= 0) & (idx <= 2 * MAXLEN - 2)
        idxc = np.clip(idx, 0, 2 * MAXLEN - 2)
        cols = rel_table[:, g * NHL:(g + 1) * NHL]   # [2047, NHL]
        at = cols[idxc] * valid[:, :, None]          # [128, 2047, NHL]
        atab_c = np.ascontiguousarray(
            at.transpose(2, 0, 1)).astype(np.float32)  # [NHL, 128, 2047]
        gdata.append((wq_c, wk_c, wv_c, wu_c, wf2_c, bq_c, bk_c, bu_c,
                      bv_c, atab_c))

    for c in range(NCORES):
        b, g = c // HGRP, c % HGRP
        (wq_c, wk_c, wv_c, wu_c, wf2_c, bq_c, bk_c, bu_c, bv_c,
         atab_c) = gdata[g]
        m = {
            "qT": np.ascontiguousarray(query[b].T).astype(npdt),
            "kT": np.ascontiguousarray(key[b].T).astype(npdt),
            "vT": np.ascontiguousarray(value[b].T).astype(npdt),
            "wq": wq_c, "wk": wk_c, "wv": wv_c, "wu": wu_c, "wf2": wf2_c,
            "bq": bq_c, "bk": bk_c, "bu": bu_c, "bv": bv_c, "atab": atab_c,
        }
        mb = attn_mask[b]
        if causal:
            # maskd[r, jb, c] = mask[i0(jb)+c, jb*128+r], i0 = (jb//4)*512
            md = np.empty((128, NHL, 512), np.float32)
            for jb in range(8):
                i0 = (jb // 4) * 512
                md[:, jb, :] = mb[i0:i0 + 512, jb * 128:(jb + 1) * 128].T
            m["maskd"] = md
        else:
            mf = np.empty((128, NHL, S), np.float32)
            for jb in range(8):
                mf[:, jb, :] = mb[:, jb * 128:(jb + 1) * 128].T
            m["maskf"] = mf
        in_maps.append(m)
    return in_maps


def kernel(query, key, value, attn_mask, Wq, bq, Wk, bk, Wv, bv, Wu, bu,
           Wf2, bf2, rel_table):
    global LAST_EXEC_NS
    query = np.asarray(query, np.float32)
    key = np.asarray(key, np.float32)
    value = np.asarray(value, np.float32)
    attn_mask = np.asarray(attn_mask, bool)
    Wq, bq = np.asarray(Wq, np.float32), np.asarray(bq, np.float32)
    Wk, bk = np.asarray(Wk, np.float32), np.asarray(bk, np.float32)
    Wv, bv = np.asarray(Wv, np.float32), np.asarray(bv, np.float32)
    Wu, bu = np.asarray(Wu, np.float32), np.asarray(bu, np.float32)
    Wf2, bf2 = np.asarray(Wf2, np.float32), np.asarray(bf2, np.float32)
    rel_table = np.asarray(rel_table, np.float32)

    tril = np.tril(np.ones((S, S), bool))
    causal = all(np.array_equal(attn_mask[b], tril) for b in range(B))

    key_ = (causal, MM_DT)
    if key_ not in _CACHE:
        _CACHE[key_] = _build(causal)
    nc = _CACHE[key_]

    in_maps = _host_shards(query, key, value, attn_mask, Wq, bq, Wk, bk,
                           Wv, bv, Wu, bu, Wf2, rel_table, causal)
    res = run_bass_kernel_spmd(nc, in_maps, list(range(NCORES)), trace=TRACE)
    if res.exec_time_ns is not None:
        LAST_EXEC_NS = res.exec_time_ns

    outp = np.empty((B, S, H), np.float32)
    for b in range(B):
        outp[b] = (res.results[2 * b]["out"] + res.results[2 * b + 1]["out"]
                   + bf2[None, :])
    return outp


# revision 6
# speedup vs baseline: 1.3706x; 1.1359x over previous
"""Trainium2 Bass kernel for nn_BaselineModel_35175782154746 (dense transformer
block with SiLU attention + relative-position bias).

Sharding: 8 NeuronCores = 4 batches x 2 head-groups (8 heads each).
Each core computes, for its (batch b, head-group g):
    U, Q, K, V projections (columns g*1024:(g+1)*1024 of Wu/Wq/Wk/Wv),
    SiLU attention with rel-pos bias for its 8 heads,
    gated = out * U, partial = gated @ Wf2[g*1024:(g+1)*1024, :].
Host reduces: out[b] = partial[2b] + partial[2b+1] + bf2.

All matmuls run as float32r (full fp32 storage, fast PE mode) with N=512
moving dim. Activations/bias layouts are arranged so the contraction dim is
always on SBUF partitions (inputs are pre-transposed on host).
"""

import sys
import os

for _p in ("/root/.axon_site/_ro/trn_rl_repo", "/opt/trn_rl_repo"):
    if os.path.isdir(_p) and _p not in sys.path:
        sys.path.append(_p)

import numpy as np

import concourse.bass as bass
import concourse.mybir as mybir
import concourse.tile as tile
from concourse import bacc
from concourse.bass_utils import run_bass_kernel_spmd

B, S, H, NH, MAXLEN = 4, 1024, 2048, 16, 1024
HD = H // NH            # 128
NHL = 8                 # heads per core (local)
HGRP = 2                # head groups
NCORES = 8
KT16 = H // 128         # 16 k-tiles for the H contraction
SCALE = float(HD) ** -0.5

f32 = mybir.dt.float32
f32r = mybir.dt.float32r
bf16 = mybir.dt.bfloat16
SILU = mybir.ActivationFunctionType.Silu
MULT = mybir.AluOpType.mult
ADD = mybir.AluOpType.add

TRACE = False
LAST_EXEC_NS = None
MM_DT = "bf16"          # "bf16" or "f32r" matmul operand dtype
_CACHE = {}


def _build(causal: bool, mm_dt=None):
    mmdt = {"bf16": bf16, "f32r": f32r}[mm_dt or MM_DT]
    nc = bacc.Bacc("TRN2", target_bir_lowering=False, debug=False,
                   num_devices=NCORES)

    def din(name, shape, dt=f32):
        return nc.dram_tensor(name, shape, dt, kind="ExternalInput").ap()

    qT = din("qT", [H, S], mmdt)
    kT = din("kT", [H, S], mmdt)
    vT = din("vT", [H, S], mmdt)
    wq = din("wq", [H, NHL * HD], mmdt)
    wk = din("wk", [H, NHL * HD], mmdt)
    wv = din("wv", [H, NHL * HD], mmdt)
    wu = din("wu", [H, NHL * HD], mmdt)
    wf2 = din("wf2", [NHL * HD, H], mmdt)
    bq = din("bq", [128, NHL])
    bk = din("bk", [128, NHL])
    bu = din("bu", [128, NHL])
    bv = din("bv", [128, NHL * HD])
    atab = din("atab", [NHL, 128, 2047])
    if causal:
        maskd = din("maskd", [128, NHL, 512])
    else:
        maskf = din("maskf", [128, NHL, S])
    out = nc.dram_tensor("out", [S, H], f32, kind="ExternalOutput").ap()

    with tile.TileContext(nc) as tc:
        with (
            tc.tile_pool(name="const", bufs=1) as constp,
            tc.tile_pool(name="gatedp", bufs=1) as gatedp,
        ):
            bq_t = constp.tile([128, NHL], f32, tag="bq")
            bk_t = constp.tile([128, NHL], f32, tag="bk")
            bu_t = constp.tile([128, NHL], f32, tag="bu")
            bv_t = constp.tile([128, NHL * HD], f32, tag="bv")
            nc.sync.dma_start(bq_t[:], bq[:])
            nc.sync.dma_start(bk_t[:], bk[:])
            nc.sync.dma_start(bu_t[:], bu[:])
            nc.sync.dma_start(bv_t[:], bv[:])

            gatedT = gatedp.tile([128, NHL, S], mmdt, tag="gatedT")

            with tc.tile_pool(name="attres", bufs=1) as attres:
                UT = attres.tile([128, NHL, S], bf16, tag="UT")
                QT = attres.tile([128, NHL, S], mmdt, tag="QT")
                KTt = attres.tile([128, NHL, S], mmdt, tag="KT")
                V = attres.tile([128, NHL, S], mmdt, tag="V")
                if causal:
                    mask_t = attres.tile([128, NHL, 512], f32, tag="mask")
                    nc.sync.dma_start(mask_t[:], maskd[:])
                else:
                    mask_t = attres.tile([128, NHL, S], f32, tag="mask")
                    nc.sync.dma_start(mask_t[:], maskf[:])

                # ---- projections U, Q, K (transposed outputs [HD, S]) ----
                with (
                    tc.tile_pool(name="xin", bufs=6) as xinp,
                    tc.tile_pool(name="win", bufs=6) as winp,
                    tc.tile_pool(name="pps", bufs=1, space="PSUM") as ppsum,
                ):
                    for wdram, indram, btile, outtile in (
                        (wu, qT, bu_t, UT),
                        (wq, qT, bq_t, QT),
                        (wk, kT, bk_t, KTt),
                    ):
                        for ih in range(2):
                            ps = [ppsum.tile([128, 512], f32, tag=f"pp{h}", name=f"pp{h}")
                                  for h in range(NHL)]
                            for k in range(KT16):
                                xt = xinp.tile([128, 512], mmdt, tag="xin")
                                nc.sync.dma_start(
                                    xt[:],
                                    indram[k * 128:(k + 1) * 128,
                                           ih * 512:(ih + 1) * 512])
                                wt = winp.tile([128, NHL * HD], mmdt, tag="win")
                                nc.sync.dma_start(
                                    wt[:], wdram[k * 128:(k + 1) * 128, :])
                                for h in range(NHL):
                                    nc.tensor.matmul(
                                        ps[h][:],
                                        lhsT=wt[:, h * HD:(h + 1) * HD],
                                        rhs=xt[:],
                                        start=(k == 0), stop=(k == KT16 - 1))
                            for h in range(NHL):
                                nc.scalar.activation(
                                    outtile[:, h, ih * 512:(ih + 1) * 512],
                                    ps[h][:], SILU, bias=btile[:, h:h + 1])

                    # ---- projection V (natural layout [S, NHL*HD]) ----
                    for ch in range(2):
                        ps = [ppsum.tile([128, 512], f32, tag=f"pp{sb}", name=f"ppv{sb}")
                              for sb in range(8)]
                        for k in range(KT16):
                            vt = xinp.tile([128, S], mmdt, tag="vin")
                            nc.sync.dma_start(
                                vt[:], vT[k * 128:(k + 1) * 128, :])
                            wt = winp.tile([128, 512], mmdt, tag="wvin")
                            nc.sync.dma_start(
                                wt[:], wv[k * 128:(k + 1) * 128,
                                          ch * 512:(ch + 1) * 512])
                            for sb in range(8):
                                nc.tensor.matmul(
                                    ps[sb][:],
                                    lhsT=vt[:, sb * 128:(sb + 1) * 128],
                                    rhs=wt[:],
                                    start=(k == 0), stop=(k == KT16 - 1))
                        for sb in range(8):
                            nc.vector.tensor_add(
                                V[:, sb, ch * 512:(ch + 1) * 512],
                                ps[sb][:],
                                bv_t[:, ch * 512:(ch + 1) * 512])
                            nc.scalar.activation(
                                V[:, sb, ch * 512:(ch + 1) * 512],
                                V[:, sb, ch * 512:(ch + 1) * 512], SILU)

                # ---- attention per head ----
                with (
                    tc.tile_pool(name="atabp", bufs=2) as atabp,
                    tc.tile_pool(name="attnp", bufs=6) as attnp,
                    tc.tile_pool(name="pssc", bufs=6, space="PSUM") as pssc,
                    tc.tile_pool(name="psav", bufs=2, space="PSUM") as psav,
                ):
                    for h in range(NHL):
                        at = atabp.tile([128, 2047], f32, tag="atab")
                        nc.sync.dma_start(at[:], atab[h])
                        for ih in range(2):
                            njb = (4 * ih + 4) if causal else 8
                            avp = psav.tile([128, 512], f32, tag="av")
                            chunks = [list(range(j, min(j + 3, njb)))
                                      for j in range(0, njb, 3)]
                            att_tiles = {}

                            def emit_scores(ch, h=h, ih=ih, at=at,
                                            att_tiles=att_tiles):
                                for jb in ch:
                                    scp = pssc.tile([128, 512], f32, tag="sc",
                                                    name=f"sc{h}_{ih}_{jb}")
                                    nc.tensor.matmul(
                                        scp[:],
                                        lhsT=KTt[:, h, jb * 128:(jb + 1) * 128],
                                        rhs=QT[:, h, ih * 512:(ih + 1) * 512],
                                        start=True, stop=True)
                                    att = attnp.tile([128, 512], mmdt,
                                                     tag="attn",
                                                     name=f"at{h}_{ih}_{jb}")
                                    d0 = ih * 512 - jb * 128 + MAXLEN - 1
                                    nc.vector.scalar_tensor_tensor(
                                        att[:], scp[:], SCALE,
                                        at[:, d0:d0 + 512],
                                        op0=MULT, op1=ADD)
                                    nc.scalar.activation(att[:], att[:], SILU)
                                    if causal:
                                        if jb // 4 == ih:
                                            nc.vector.tensor_mul(
                                                att[:], att[:],
                                                mask_t[:, jb, :])
                                    else:
                                        nc.vector.tensor_mul(
                                            att[:], att[:],
                                            mask_t[:, jb,
                                                   ih * 512:(ih + 1) * 512])
                                    att_tiles[jb] = att

                            emit_scores(chunks[0])
                            for ci, ch in enumerate(chunks):
                                if ci + 1 < len(chunks):
                                    emit_scores(chunks[ci + 1])
                                for jb in ch:
                                    nc.tensor.matmul(
                                        avp[:],
                                        lhsT=V[:, jb, h * HD:(h + 1) * HD],
                                        rhs=att_tiles.pop(jb)[:],
                                        start=(jb == 0), stop=(jb == njb - 1))
                            nc.vector.tensor_mul(
                                gatedT[:, h, ih * 512:(ih + 1) * 512],
                                avp[:],
                                UT[:, h, ih * 512:(ih + 1) * 512])

            # ---- f2: partial = gatedT.T @ Wf2 (bf2 added on host) ----
            with (
                tc.tile_pool(name="wf2p", bufs=2) as wf2p,
                tc.tile_pool(name="stgp", bufs=3) as stgp,
                tc.tile_pool(name="psf2", bufs=4, space="PSUM") as psf2,
            ):
                wf2r = wf2.rearrange("(cb p) n -> p cb n", p=128)
                for n in range(4):
                    w2 = wf2p.tile([128, NHL, 512], mmdt, tag="wf2")
                    nc.sync.dma_start(w2[:], wf2r[:, :, n * 512:(n + 1) * 512])
                    for sb in range(8):
                        ps = psf2.tile([128, 512], f32, tag="f2")
                        for cb in range(NHL):
                            nc.tensor.matmul(
                                ps[:],
                                lhsT=gatedT[:, cb, sb * 128:(sb + 1) * 128],
                                rhs=w2[:, cb, :],
                                start=(cb == 0), stop=(cb == NHL - 1))
                        st = stgp.tile([128, 512], f32, tag="st")
                        nc.vector.tensor_copy(st[:], ps[:])
                        nc.sync.dma_start(
                            out[sb * 128:(sb + 1) * 128,
                                n * 512:(n + 1) * 512], st[:])

    nc.compile()
    return nc


def _host_shards(query, key, value, attn_mask, Wq, bq, Wk, bk, Wv, bv,
                 Wu, bu, Wf2, rel_table, causal, mm_dt=None):
    """Build the per-core input maps."""
    import ml_dtypes
    npdt = (np.dtype(ml_dtypes.bfloat16) if (mm_dt or MM_DT) == "bf16"
            else np.float32)
    in_maps = []
    # precompute per-head-group weight slices once (shared by 4 cores each)
    gdata = []
    for g in range(HGRP):
        c0, c1 = g * NHL * HD, (g + 1) * NHL * HD
        wq_c = np.ascontiguousarray(Wq[:, c0:c1]).astype(npdt)
        wk_c = np.ascontiguousarray(Wk[:, c0:c1]).astype(npdt)
        wv_c = np.ascontiguousarray(Wv[:, c0:c1]).astype(npdt)
        wu_c = np.ascontiguousarray(Wu[:, c0:c1]).astype(npdt)
        wf2_c = np.ascontiguousarray(Wf2[c0:c1, :]).astype(npdt)
        bq_c = np.ascontiguousarray(bq[c0:c1].reshape(NHL, 128).T)
        bk_c = np.ascontiguousarray(bk[c0:c1].reshape(NHL, 128).T)
        bu_c = np.ascontiguousarray(bu[c0:c1].reshape(NHL, 128).T)
        bv_c = np.ascontiguousarray(
            np.broadcast_to(bv[c0:c1][None, :], (128, NHL * HD)))
        # atab[h, r, y] = rel_table[y - r, g*NHL + h]  (0 where y < r)
        y = np.arange(2047)[None, :]
        r = np.arange(128)[:, None]
        idx = y - r                      # [128, 2047]
        valid = (idx >= 0) & (idx <= 2 * MAXLEN - 2)
        idxc = np.clip(idx, 0, 2 * MAXLEN - 2)
        cols = rel_table[:, g * NHL:(g + 1) * NHL]   # [2047, NHL]
        at = cols[idxc] * valid[:, :, None]          # [128, 2047, NHL]
        atab_c = np.ascontiguousarray(
            at.transpose(2, 0, 1)).astype(np.float32)  # [NHL, 128, 2047]
        gdata.append((wq_c, wk_c, wv_c, wu_c, wf2_c, bq_c, bk_c, bu_c,
                      bv_c, atab_c))

    for c in range(NCORES):
        b, g = c // HGRP, c % HGRP
        (wq_c, wk_c, wv_c, wu_c, wf2_c, bq_c, bk_c, bu_c, bv_c,
         atab_c) = gdata[g]
        m = {
            "qT": np.ascontiguousarray(query[b].T).astype(npdt),
            "kT": np.ascontiguousarray(key[b].T).astype(npdt),
            "vT": np.ascontiguousarray(value[b].T).astype(npdt),
            "wq": wq_c, "wk": wk_c, "wv": wv_c, "wu": wu_c, "wf2": wf2_c,
            "bq": bq_c, "bk": bk_c, "bu": bu_c, "bv": bv_c, "atab": atab_c,
        }
        mb = attn_mask[b]
        if causal:
            # maskd[r, jb, c] = mask[i0(jb)+c, jb*128+r], i0 = (jb//4)*512
            md = np.empty((128, NHL, 512), np.float32)
            for jb in range(8):
                i0 = (jb // 4) * 512
                md[:, jb, :] = mb[i0:i0 + 512, jb * 128:(jb + 1) * 128].T
            m["maskd"] = md
        else:
            mf = np.empty((128, NHL, S), np.float32)
            for jb in range(8):
                mf[:, jb, :] = mb[:, jb * 128:(jb + 1) * 128].T
            m["maskf"] = mf
        in_maps.append(m)
    return in_maps


def kernel(query, key, value, attn_mask, Wq, bq, Wk, bk, Wv, bv, Wu, bu,
           Wf2, bf2, rel_table):
    global LAST_EXEC_NS
    query = np.asarray(query, np.float32)
    key = np.asarray(key, np.float32)
    value = np.asarray(value, np.float32)
    attn_mask = np.asarray(attn_mask, bool)
    Wq, bq = np.asarray(Wq, np.float32), np.asarray(bq, np.float32)
    Wk, bk = np.asarray(Wk, np.float32), np.asarray(bk, np.float32)
    Wv, bv = np.asarray(Wv, np.float32), np.asarray(bv, np.float32)
    Wu, bu = np.asarray(Wu, np.float32), np.asarray(bu, np.float32)
    Wf2, bf2 = np.asarray(Wf2, np.float32), np.asarray(bf2, np.float32)
    rel_table = np.asarray(rel_table, np.float32)

    tril = np.tril(np.ones((S, S), bool))
    causal = all(np.array_equal(attn_mask[b], tril) for b in range(B))

    key_ = (causal, MM_DT)
    if key_ not in _CACHE:
        _CACHE[key_] = _build(causal)
    nc = _CACHE[key_]

    in_maps = _host_shards(query, key, value, attn_mask, Wq, bq, Wk, bk,
                           Wv, bv, Wu, bu, Wf2, rel_table, causal)
    res = run_bass_kernel_spmd(nc, in_maps, list(range(NCORES)), trace=TRACE)
    if res.exec_time_ns is not None:
        LAST_EXEC_NS = res.exec_time_ns

    outp = np.empty((B, S, H), np.float32)
    for b in range(B):
        outp[b] = (res.results[2 * b]["out"] + res.results[2 * b + 1]["out"]
                   + bf2[None, :])
    return outp


# revision 8
# speedup vs baseline: 1.5363x; 1.1209x over previous
"""Trainium2 Bass kernel for nn_BaselineModel_35175782154746 (dense transformer
block with SiLU attention + relative-position bias).

Sharding: 8 NeuronCores = 4 batches x 2 head-groups (8 heads each).
Each core computes, for its (batch b, head-group g):
    U, Q, K, V projections (columns g*1024:(g+1)*1024 of Wu/Wq/Wk/Wv),
    SiLU attention with rel-pos bias for its 8 heads,
    gated = out * U, partial = gated @ Wf2[g*1024:(g+1)*1024, :].
Host reduces: out[b] = partial[2b] + partial[2b+1] + bf2.

All matmuls run as float32r (full fp32 storage, fast PE mode) with N=512
moving dim. Activations/bias layouts are arranged so the contraction dim is
always on SBUF partitions (inputs are pre-transposed on host).
"""

import sys
import os

for _p in ("/root/.axon_site/_ro/trn_rl_repo", "/opt/trn_rl_repo"):
    if os.path.isdir(_p) and _p not in sys.path:
        sys.path.append(_p)

import numpy as np

import concourse.bass as bass
import concourse.mybir as mybir
import concourse.tile as tile
from concourse import bacc
from concourse.bass_utils import run_bass_kernel_spmd

B, S, H, NH, MAXLEN = 4, 1024, 2048, 16, 1024
HD = H // NH            # 128
NHL = 8                 # heads per core (local)
HGRP = 2                # head groups
NCORES = 8
KT16 = H // 128         # 16 k-tiles for the H contraction
SCALE = float(HD) ** -0.5

f32 = mybir.dt.float32
f32r = mybir.dt.float32r
bf16 = mybir.dt.bfloat16
SILU = mybir.ActivationFunctionType.Silu
MULT = mybir.AluOpType.mult
ADD = mybir.AluOpType.add

TRACE = False
LAST_EXEC_NS = None
MM_DT = "bf16"          # "bf16" or "f32r" matmul operand dtype
_CACHE = {}


def _build(causal: bool, mm_dt=None):
    mmdt = {"bf16": bf16, "f32r": f32r}[mm_dt or MM_DT]
    nc = bacc.Bacc("TRN2", target_bir_lowering=False, debug=False,
                   num_devices=NCORES)

    def din(name, shape, dt=f32):
        return nc.dram_tensor(name, shape, dt, kind="ExternalInput").ap()

    qT = din("qT", [H, S], mmdt)
    kT = din("kT", [H, S], mmdt)
    vT = din("vT", [H, S], mmdt)
    wq = din("wq", [H, NHL * HD], mmdt)
    wk = din("wk", [H, NHL * HD], mmdt)
    wv = din("wv", [H, NHL * HD], mmdt)
    wu = din("wu", [H, NHL * HD], mmdt)
    wf2 = din("wf2", [NHL * HD, H], mmdt)
    bq = din("bq", [128, NHL])
    bk = din("bk", [128, NHL])
    bu = din("bu", [128, NHL])
    bv = din("bv", [1, NHL * HD], mmdt)
    ones1 = din("ones1", [1, 128], mmdt)
    if causal:
        atab = din("atab", [NHL, 128, 2047], mmdt)
        ident = din("ident", [128, 128], mmdt)
    else:
        atab = din("atab", [NHL, 128, 2047])
        maskf = din("maskf", [128, NHL, S])
    out = nc.dram_tensor("out", [S, H], f32, kind="ExternalOutput").ap()

    with tile.TileContext(nc) as tc:
        with (
            tc.tile_pool(name="const", bufs=1) as constp,
            tc.tile_pool(name="gatedp", bufs=1) as gatedp,
        ):
            bq_t = constp.tile([128, NHL], f32, tag="bq")
            bk_t = constp.tile([128, NHL], f32, tag="bk")
            bu_t = constp.tile([128, NHL], f32, tag="bu")
            bv_t = constp.tile([1, NHL * HD], mmdt, tag="bv")
            ones_t = constp.tile([1, 128], mmdt, tag="ones1")
            nc.sync.dma_start(ones_t[:], ones1[:])
            if causal:
                id_t = constp.tile([128, 128], mmdt, tag="ident")
                nc.sync.dma_start(id_t[:], ident[:])
            nc.sync.dma_start(bq_t[:], bq[:])
            nc.sync.dma_start(bk_t[:], bk[:])
            nc.sync.dma_start(bu_t[:], bu[:])
            nc.sync.dma_start(bv_t[:], bv[:])

            gatedT = gatedp.tile([128, NHL, S], mmdt, tag="gatedT")

            with tc.tile_pool(name="attres", bufs=1) as attres:
                UT = attres.tile([128, NHL, S], bf16, tag="UT")
                QT = attres.tile([128, NHL, S], mmdt, tag="QT")
                KTt = attres.tile([128, NHL, S], mmdt, tag="KT")
                V = attres.tile([128, NHL, S], mmdt, tag="V")
                if not causal:
                    mask_t = attres.tile([128, NHL, S], f32, tag="mask")
                    nc.sync.dma_start(mask_t[:], maskf[:])

                # ---- projections U, Q, K (transposed outputs [HD, S]) ----
                with (
                    tc.tile_pool(name="xin", bufs=6) as xinp,
                    tc.tile_pool(name="win", bufs=6) as winp,
                    tc.tile_pool(name="pps", bufs=1, space="PSUM") as ppsum,
                ):
                    for wdram, indram, btile, outtile in (
                        (wu, qT, bu_t, UT),
                        (wq, qT, bq_t, QT),
                        (wk, kT, bk_t, KTt),
                    ):
                        for ih in range(2):
                            ps = [ppsum.tile([128, 512], f32, tag=f"pp{h}", name=f"pp{h}")
                                  for h in range(NHL)]
                            for k in range(KT16):
                                xt = xinp.tile([128, 512], mmdt, tag="xin")
                                nc.sync.dma_start(
                                    xt[:],
                                    indram[k * 128:(k + 1) * 128,
                                           ih * 512:(ih + 1) * 512])
                                wt = winp.tile([128, NHL * HD], mmdt, tag="win")
                                nc.sync.dma_start(
                                    wt[:], wdram[k * 128:(k + 1) * 128, :])
                                for h in range(NHL):
                                    nc.tensor.matmul(
                                        ps[h][:],
                                        lhsT=wt[:, h * HD:(h + 1) * HD],
                                        rhs=xt[:],
                                        start=(k == 0), stop=(k == KT16 - 1))
                            for h in range(NHL):
                                nc.scalar.activation(
                                    outtile[:, h, ih * 512:(ih + 1) * 512],
                                    ps[h][:], SILU, bias=btile[:, h:h + 1])

                    # ---- projection V (natural layout [S, NHL*HD]) ----
                    for ch in range(2):
                        ps = [ppsum.tile([128, 512], f32, tag=f"pp{sb}", name=f"ppv{sb}")
                              for sb in range(8)]
                        for k in range(KT16):
                            vt = xinp.tile([128, S], mmdt, tag="vin")
                            nc.sync.dma_start(
                                vt[:], vT[k * 128:(k + 1) * 128, :])
                            wt = winp.tile([128, 512], mmdt, tag="wvin")
                            nc.sync.dma_start(
                                wt[:], wv[k * 128:(k + 1) * 128,
                                          ch * 512:(ch + 1) * 512])
                            for sb in range(8):
                                nc.tensor.matmul(
                                    ps[sb][:],
                                    lhsT=vt[:, sb * 128:(sb + 1) * 128],
                                    rhs=wt[:],
                                    start=(k == 0), stop=False)
                        for sb in range(8):
                            nc.tensor.matmul(
                                ps[sb][:],
                                lhsT=ones_t[:],
                                rhs=bv_t[:, ch * 512:(ch + 1) * 512],
                                start=False, stop=True)
                            nc.scalar.activation(
                                V[:, sb, ch * 512:(ch + 1) * 512],
                                ps[sb][:], SILU)

                # ---- attention per head ----
                with (
                    tc.tile_pool(name="atabp", bufs=2) as atabp,
                    tc.tile_pool(name="attnp", bufs=6) as attnp,
                    tc.tile_pool(name="pssc", bufs=6, space="PSUM") as pssc,
                    tc.tile_pool(name="psav", bufs=2, space="PSUM") as psav,
                ):
                    for h in range(NHL):
                        at = atabp.tile([128, 2047], mmdt if causal else f32, tag="atab")
                        nc.sync.dma_start(at[:], atab[h])
                        for ih in range(2):
                            njb = (4 * ih + 4) if causal else 8
                            avp = psav.tile([128, 512], f32, tag="av")
                            chunks = [list(range(j, min(j + 3, njb)))
                                      for j in range(0, njb, 3)]
                            att_tiles = {}

                            def emit_scores(ch, h=h, ih=ih, at=at,
                                            att_tiles=att_tiles):
                                for jb in ch:
                                    scp = pssc.tile([128, 512], f32, tag="sc",
                                                    name=f"sc{h}_{ih}_{jb}")
                                    nc.tensor.matmul(
                                        scp[:],
                                        lhsT=KTt[:, h, jb * 128:(jb + 1) * 128],
                                        rhs=QT[:, h, ih * 512:(ih + 1) * 512],
                                        start=True, stop=not causal)
                                    att = attnp.tile([128, 512], mmdt,
                                                     tag="attn",
                                                     name=f"at{h}_{ih}_{jb}")
                                    d0 = ih * 512 - jb * 128 + MAXLEN - 1
                                    if causal:
                                        # scores += rel_bias/scale (mask encoded
                                        # as -1e5 in the table), then
                                        # att = silu(scale * psum) on ACT.
                                        nc.tensor.matmul(
                                            scp[:], lhsT=id_t[:],
                                            rhs=at[:, d0:d0 + 512],
                                            start=False, stop=True)
                                        nc.scalar.activation(
                                            att[:], scp[:], SILU, scale=SCALE)
                                    else:
                                        nc.vector.scalar_tensor_tensor(
                                            att[:], scp[:], SCALE,
                                            at[:, d0:d0 + 512],
                                            op0=MULT, op1=ADD)
                                        nc.scalar.activation(att[:], att[:],
                                                             SILU)
                                        nc.vector.tensor_mul(
                                            att[:], att[:],
                                            mask_t[:, jb,
                                                   ih * 512:(ih + 1) * 512])
                                    att_tiles[jb] = att

                            emit_scores(chunks[0])
                            for ci, ch in enumerate(chunks):
                                if ci + 1 < len(chunks):
                                    emit_scores(chunks[ci + 1])
                                for jb in ch:
                                    nc.tensor.matmul(
                                        avp[:],
                                        lhsT=V[:, jb, h * HD:(h + 1) * HD],
                                        rhs=att_tiles.pop(jb)[:],
                                        start=(jb == 0), stop=(jb == njb - 1))
                            nc.vector.tensor_mul(
                                gatedT[:, h, ih * 512:(ih + 1) * 512],
                                avp[:],
                                UT[:, h, ih * 512:(ih + 1) * 512])

            # ---- f2: partial = gatedT.T @ Wf2 (bf2 added on host) ----
            with (
                tc.tile_pool(name="wf2p", bufs=2) as wf2p,
                tc.tile_pool(name="stgp", bufs=3) as stgp,
                tc.tile_pool(name="psf2", bufs=4, space="PSUM") as psf2,
            ):
                wf2r = wf2.rearrange("(cb p) n -> p cb n", p=128)
                for n in range(4):
                    w2 = wf2p.tile([128, NHL, 512], mmdt, tag="wf2")
                    nc.sync.dma_start(w2[:], wf2r[:, :, n * 512:(n + 1) * 512])
                    for sb in range(8):
                        ps = psf2.tile([128, 512], f32, tag="f2")
                        for cb in range(NHL):
                            nc.tensor.matmul(
                                ps[:],
                                lhsT=gatedT[:, cb, sb * 128:(sb + 1) * 128],
                                rhs=w2[:, cb, :],
                                start=(cb == 0), stop=(cb == NHL - 1))
                        st = stgp.tile([128, 512], f32, tag="st")
                        nc.vector.tensor_copy(st[:], ps[:])
                        nc.sync.dma_start(
                            out[sb * 128:(sb + 1) * 128,
                                n * 512:(n + 1) * 512], st[:])

    nc.compile()
    return nc


def _host_shards(query, key, value, attn_mask, Wq, bq, Wk, bk, Wv, bv,
                 Wu, bu, Wf2, rel_table, causal, mm_dt=None):
    """Build the per-core input maps."""
    import ml_dtypes
    npdt = (np.dtype(ml_dtypes.bfloat16) if (mm_dt or MM_DT) == "bf16"
            else np.float32)
    _EYE128 = np.eye(128).astype(npdt)
    _ONES128 = np.ones((1, 128)).astype(npdt)
    in_maps = []
    # precompute per-head-group weight slices once (shared by 4 cores each)
    gdata = []
    for g in range(HGRP):
        c0, c1 = g * NHL * HD, (g + 1) * NHL * HD
        wq_c = np.ascontiguousarray(Wq[:, c0:c1]).astype(npdt)
        wk_c = np.ascontiguousarray(Wk[:, c0:c1]).astype(npdt)
        wv_c = np.ascontiguousarray(Wv[:, c0:c1]).astype(npdt)
        wu_c = np.ascontiguousarray(Wu[:, c0:c1]).astype(npdt)
        wf2_c = np.ascontiguousarray(Wf2[c0:c1, :]).astype(npdt)
        bq_c = np.ascontiguousarray(bq[c0:c1].reshape(NHL, 128).T)
        bk_c = np.ascontiguousarray(bk[c0:c1].reshape(NHL, 128).T)
        bu_c = np.ascontiguousarray(bu[c0:c1].reshape(NHL, 128).T)
        bv_c = np.ascontiguousarray(bv[c0:c1][None, :]).astype(npdt)
        # atab[h, r, y] = table[y - r, g*NHL + h]; for the causal variant the
        # table is pre-divided by SCALE and masked entries (m < MAXLEN-1,
        # i.e. key index > query index) are -1e5 so silu gives exactly 0.
        y = np.arange(2047)[None, :]
        r = np.arange(128)[:, None]
        idx = y - r                      # [128, 2047]
        valid = (idx >= 0) & (idx <= 2 * MAXLEN - 2)
        idxc = np.clip(idx, 0, 2 * MAXLEN - 2)
        cols = rel_table[:, g * NHL:(g + 1) * NHL]   # [2047, NHL]
        if causal:
            cols = cols / np.float32(SCALE)
            cols = np.where(np.arange(2047)[:, None] >= MAXLEN - 1, cols,
                            np.float32(-1e5))
            at = np.where(valid[:, :, None], cols[idxc], np.float32(-1e5))
            atab_c = np.ascontiguousarray(at.transpose(2, 0, 1)).astype(npdt)
        else:
            at = cols[idxc] * valid[:, :, None]
            atab_c = np.ascontiguousarray(
                at.transpose(2, 0, 1)).astype(np.float32)
        gdata.append((wq_c, wk_c, wv_c, wu_c, wf2_c, bq_c, bk_c, bu_c,
                      bv_c, atab_c))

    for c in range(NCORES):
        b, g = c // HGRP, c % HGRP
        (wq_c, wk_c, wv_c, wu_c, wf2_c, bq_c, bk_c, bu_c, bv_c,
         atab_c) = gdata[g]
        m = {
            "qT": np.ascontiguousarray(query[b].T).astype(npdt),
            "kT": np.ascontiguousarray(key[b].T).astype(npdt),
            "vT": np.ascontiguousarray(value[b].T).astype(npdt),
            "wq": wq_c, "wk": wk_c, "wv": wv_c, "wu": wu_c, "wf2": wf2_c,
            "bq": bq_c, "bk": bk_c, "bu": bu_c, "bv": bv_c, "atab": atab_c,
            "ones1": _ONES128,
        }
        mb = attn_mask[b]
        if causal:
            m["ident"] = _EYE128
        else:
            mf = np.empty((128, NHL, S), np.float32)
            for jb in range(8):
                mf[:, jb, :] = mb[:, jb * 128:(jb + 1) * 128].T
            m["maskf"] = mf
        in_maps.append(m)
    return in_maps


def kernel(query, key, value, attn_mask, Wq, bq, Wk, bk, Wv, bv, Wu, bu,
           Wf2, bf2, rel_table):
    global LAST_EXEC_NS
    query = np.asarray(query, np.float32)
    key = np.asarray(key, np.float32)
    value = np.asarray(value, np.float32)
    attn_mask = np.asarray(attn_mask, bool)
    Wq, bq = np.asarray(Wq, np.float32), np.asarray(bq, np.float32)
    Wk, bk = np.asarray(Wk, np.float32), np.asarray(bk, np.float32)
    Wv, bv = np.asarray(Wv, np.float32), np.asarray(bv, np.float32)
    Wu, bu = np.asarray(Wu, np.float32), np.asarray(bu, np.float32)
    Wf2, bf2 = np.asarray(Wf2, np.float32), np.asarray(bf2, np.float32)
    rel_table = np.asarray(rel_table, np.float32)

    tril = np.tril(np.ones((S, S), bool))
    causal = all(np.array_equal(attn_mask[b], tril) for b in range(B))

    key_ = (causal, MM_DT)
    if key_ not in _CACHE:
        _CACHE[key_] = _build(causal)
    nc = _CACHE[key_]

    in_maps = _host_shards(query, key, value, attn_mask, Wq, bq, Wk, bk,
                           Wv, bv, Wu, bu, Wf2, rel_table, causal)
    res = run_bass_kernel_spmd(nc, in_maps, list(range(NCORES)), trace=TRACE)
    if res.exec_time_ns is not None:
        LAST_EXEC_NS = res.exec_time_ns

    outp = np.empty((B, S, H), np.float32)
    for b in range(B):
        outp[b] = (res.results[2 * b]["out"] + res.results[2 * b + 1]["out"]
                   + bf2[None, :])
    return outp


# revision 12
# speedup vs baseline: 1.5569x; 1.0134x over previous
"""Trainium2 Bass kernel for nn_BaselineModel_35175782154746 (dense transformer
block with SiLU attention + relative-position bias).

Sharding: 8 NeuronCores = 4 batches x 2 head-groups (8 heads each).
Each core computes, for its (batch b, head-group g):
    U, Q, K, V projections (columns g*1024:(g+1)*1024 of Wu/Wq/Wk/Wv),
    SiLU attention with rel-pos bias for its 8 heads,
    gated = out * U, partial = gated @ Wf2[g*1024:(g+1)*1024, :].
Host reduces: out[b] = partial[2b] + partial[2b+1] + bf2.

All matmuls run as float32r (full fp32 storage, fast PE mode) with N=512
moving dim. Activations/bias layouts are arranged so the contraction dim is
always on SBUF partitions (inputs are pre-transposed on host).
"""

import sys
import os

for _p in ("/root/.axon_site/_ro/trn_rl_repo", "/opt/trn_rl_repo"):
    if os.path.isdir(_p) and _p not in sys.path:
        sys.path.append(_p)

import numpy as np

import concourse.bass as bass
import concourse.mybir as mybir
import concourse.tile as tile
from concourse import bacc
from concourse.bass_utils import run_bass_kernel_spmd

B, S, H, NH, MAXLEN = 4, 1024, 2048, 16, 1024
HD = H // NH            # 128
NHL = 8                 # heads per core (local)
HGRP = 2                # head groups
NCORES = 8
KT16 = H // 128         # 16 k-tiles for the H contraction
SCALE = float(HD) ** -0.5

f32 = mybir.dt.float32
f32r = mybir.dt.float32r
bf16 = mybir.dt.bfloat16
SILU = mybir.ActivationFunctionType.Silu
MULT = mybir.AluOpType.mult
ADD = mybir.AluOpType.add

TRACE = False
LAST_EXEC_NS = None
MM_DT = "bf16"          # "bf16" or "f32r" matmul operand dtype
_CACHE = {}


def _build(causal: bool, mm_dt=None):
    mmdt = {"bf16": bf16, "f32r": f32r}[mm_dt or MM_DT]
    nc = bacc.Bacc("TRN2", target_bir_lowering=False, debug=False,
                   num_devices=NCORES)

    def din(name, shape, dt=f32):
        return nc.dram_tensor(name, shape, dt, kind="ExternalInput").ap()

    qT = din("qT", [H, S], mmdt)
    kT = din("kT", [H, S], mmdt)
    vT = din("vT", [H, S], mmdt)
    wq = din("wq", [H, NHL * HD], mmdt)
    wk = din("wk", [H, NHL * HD], mmdt)
    wv = din("wv", [H, NHL * HD], mmdt)
    wu = din("wu", [H, NHL * HD], mmdt)
    wf2 = din("wf2", [NHL * HD, H], mmdt)
    bq = din("bq", [128, NHL])
    bk = din("bk", [128, NHL])
    bu = din("bu", [128, NHL])
    bv = din("bv", [1, NHL * HD], mmdt)
    ones1 = din("ones1", [1, 128], mmdt)
    if causal:
        atab = din("atab", [NHL, 128, 2047], mmdt)
        ident = din("ident", [128, 128], mmdt)
    else:
        atab = din("atab", [NHL, 128, 2047])
        maskf = din("maskf", [128, NHL, S], bf16)
    out = nc.dram_tensor("out", [S, H], f32, kind="ExternalOutput").ap()

    with tile.TileContext(nc) as tc:
        with (
            tc.tile_pool(name="const", bufs=1) as constp,
            tc.tile_pool(name="gatedp", bufs=1) as gatedp,
        ):
            bq_t = constp.tile([128, NHL], f32, tag="bq")
            bk_t = constp.tile([128, NHL], f32, tag="bk")
            bu_t = constp.tile([128, NHL], f32, tag="bu")
            bv_t = constp.tile([1, NHL * HD], mmdt, tag="bv")
            ones_t = constp.tile([1, 128], mmdt, tag="ones1")
            nc.sync.dma_start(bq_t[:], bq[:])
            nc.sync.dma_start(bk_t[:], bk[:])
            nc.sync.dma_start(bu_t[:], bu[:])
            nc.sync.dma_start(bv_t[:], bv[:])
            nc.sync.dma_start(ones_t[:], ones1[:])
            if causal:
                id_t = constp.tile([128, 128], mmdt, tag="ident")
                nc.sync.dma_start(id_t[:], ident[:])

            gatedT = gatedp.tile([128, NHL, S], mmdt, tag="gatedT")
            w2_tiles = [gatedp.tile([128, NHL, 512], mmdt, tag=f"wf2_{n % 2}",
                                    name=f"wf2_{n}") for n in range(4)]

            with (
                tc.tile_pool(name="inres", bufs=1) as inres,
                tc.tile_pool(name="attres", bufs=1) as attres,
            ):
                qres = inres.tile([128, KT16, S], mmdt, tag="qres")
                kres = inres.tile([128, KT16, S], mmdt, tag="kres")
                vres = inres.tile([128, KT16, S], mmdt, tag="qres", name="vres")
                # split per-k loads so the first sweep starts after one chunk
                for k in range(KT16):
                    nc.sync.dma_start(qres[:, k, :], qT[k * 128:(k + 1) * 128, :])
                for k in range(KT16):
                    nc.sync.dma_start(kres[:, k, :], kT[k * 128:(k + 1) * 128, :])
                for k in range(KT16):
                    nc.sync.dma_start(vres[:, k, :], vT[k * 128:(k + 1) * 128, :])

                UT = attres.tile([128, NHL, S], bf16, tag="UT")
                QT = attres.tile([128, NHL, S], mmdt, tag="QT")
                KTt = attres.tile([128, NHL, S], mmdt, tag="KT")
                V = attres.tile([128, NHL, S], mmdt, tag="V")
                at_tiles = [attres.tile([128, 2047], mmdt if causal else f32,
                                        tag=f"atab{h % (4 if causal else 2)}", name=f"atab{h}")
                            for h in range(NHL)]
                if not causal:
                    mask_t = attres.tile([128, NHL, S], bf16, tag="mask")
                    nc.sync.dma_start(mask_t[:], maskf[:])
                # prefetched during the projection phases (sync queue, after
                # the input loads)
                for h in range(NHL):
                    nc.sync.dma_start(at_tiles[h][:], atab[h])
                wf2r = wf2.rearrange("(cb p) n -> p cb n", p=128)
                for n in range(4):
                    nc.sync.dma_start(w2_tiles[n][:],
                                      wf2r[:, :, n * 512:(n + 1) * 512])

                # ---- projections U, Q, K (transposed outputs [HD, S]) ----
                with (
                    tc.tile_pool(name="win", bufs=6) as winp,
                    tc.tile_pool(name="pps", bufs=1, space="PSUM") as ppsum,
                ):
                    for wdram, xres, btile, outtile in (
                        (wu, qres, bu_t, UT),
                        (wq, qres, bq_t, QT),
                        (wk, kres, bk_t, KTt),
                    ):
                        for ih in range(2):
                            ps = [ppsum.tile([128, 512], f32, tag=f"pp{h}",
                                             name=f"pp{h}")
                                  for h in range(NHL)]
                            for k in range(KT16):
                                wt = winp.tile([128, NHL * HD], mmdt, tag="win")
                                nc.gpsimd.dma_start(
                                    wt[:], wdram[k * 128:(k + 1) * 128, :])
                                for h in range(NHL):
                                    nc.tensor.matmul(
                                        ps[h][:],
                                        lhsT=wt[:, h * HD:(h + 1) * HD],
                                        rhs=qres[:, k, ih * 512:(ih + 1) * 512]
                                        if xres is qres
                                        else kres[:, k, ih * 512:(ih + 1) * 512],
                                        start=(k == 0), stop=(k == KT16 - 1))
                            for h in range(NHL):
                                nc.scalar.activation(
                                    outtile[:, h, ih * 512:(ih + 1) * 512],
                                    ps[h][:], SILU, bias=btile[:, h:h + 1])

                    # ---- projection V (natural layout [S, NHL*HD]) ----
                    for ch in range(2):
                        ps = [ppsum.tile([128, 512], f32, tag=f"pp{sb}",
                                         name=f"ppv{sb}")
                              for sb in range(8)]
                        for k in range(KT16):
                            wt = winp.tile([128, 512], mmdt, tag="wvin")
                            nc.gpsimd.dma_start(
                                wt[:], wv[k * 128:(k + 1) * 128,
                                          ch * 512:(ch + 1) * 512])
                            for sb in range(8):
                                nc.tensor.matmul(
                                    ps[sb][:],
                                    lhsT=vres[:, k, sb * 128:(sb + 1) * 128],
                                    rhs=wt[:],
                                    start=(k == 0), stop=False)
                        for sb in range(8):
                            nc.tensor.matmul(
                                ps[sb][:],
                                lhsT=ones_t[:],
                                rhs=bv_t[:, ch * 512:(ch + 1) * 512],
                                start=False, stop=True)
                            nc.scalar.activation(
                                V[:, sb, ch * 512:(ch + 1) * 512],
                                ps[sb][:], SILU)

                # ---- attention per head ----
                with (
                    tc.tile_pool(name="attnp", bufs=6) as attnp,
                    tc.tile_pool(name="pssc", bufs=6, space="PSUM") as pssc,
                    tc.tile_pool(name="psav", bufs=2, space="PSUM") as psav,
                ):
                    for h in range(NHL):
                        at = at_tiles[h]
                        for ih in range(2):
                            njb = (4 * ih + 4) if causal else 8
                            avp = psav.tile([128, 512], f32, tag="av")
                            chunks = [list(range(j, min(j + 3, njb)))
                                      for j in range(0, njb, 3)]
                            att_tiles = {}

                            def emit_scores(ch_, h=h, ih=ih, at=at,
                                            att_tiles=att_tiles):
                                for jb in ch_:
                                    scp = pssc.tile([128, 512], f32, tag="sc",
                                                    name=f"sc{h}_{ih}_{jb}")
                                    nc.tensor.matmul(
                                        scp[:],
                                        lhsT=KTt[:, h, jb * 128:(jb + 1) * 128],
                                        rhs=QT[:, h, ih * 512:(ih + 1) * 512],
                                        start=True, stop=not causal)
                                    att = attnp.tile([128, 512], mmdt,
                                                     tag="attn",
                                                     name=f"at{h}_{ih}_{jb}")
                                    d0 = ih * 512 - jb * 128 + MAXLEN - 1
                                    if causal:
                                        nc.tensor.matmul(
                                            scp[:], lhsT=id_t[:],
                                            rhs=at[:, d0:d0 + 512],
                                            start=False, stop=True)
                                        nc.scalar.activation(
                                            att[:], scp[:], SILU, scale=SCALE)
                                    else:
                                        nc.vector.scalar_tensor_tensor(
                                            att[:], scp[:], SCALE,
                                            at[:, d0:d0 + 512],
                                            op0=MULT, op1=ADD)
                                        nc.scalar.activation(att[:], att[:],
                                                             SILU)
                                        nc.vector.tensor_mul(
                                            att[:], att[:],
                                            mask_t[:, jb,
                                                   ih * 512:(ih + 1) * 512])
                                    att_tiles[jb] = att

                            emit_scores(chunks[0])
                            for ci, ch_ in enumerate(chunks):
                                if ci + 1 < len(chunks):
                                    emit_scores(chunks[ci + 1])
                                for jb in ch_:
                                    nc.tensor.matmul(
                                        avp[:],
                                        lhsT=V[:, jb, h * HD:(h + 1) * HD],
                                        rhs=att_tiles.pop(jb)[:],
                                        start=(jb == 0), stop=(jb == njb - 1))
                            nc.vector.tensor_mul(
                                gatedT[:, h, ih * 512:(ih + 1) * 512],
                                avp[:],
                                UT[:, h, ih * 512:(ih + 1) * 512])

            # ---- f2: partial = gatedT.T @ Wf2 (bf2 added on host) ----
            with (
                tc.tile_pool(name="stgp", bufs=3) as stgp,
                tc.tile_pool(name="psf2", bufs=4, space="PSUM") as psf2,
            ):
                for n in range(4):
                    w2 = w2_tiles[n]
                    for sb in range(8):
                        ps = psf2.tile([128, 512], f32, tag="f2")
                        for cb in range(NHL):
                            nc.tensor.matmul(
                                ps[:],
                                lhsT=gatedT[:, cb, sb * 128:(sb + 1) * 128],
                                rhs=w2[:, cb, :],
                                start=(cb == 0), stop=(cb == NHL - 1))
                        st = stgp.tile([128, 512], f32, tag="st")
                        nc.vector.tensor_copy(st[:], ps[:])
                        nc.sync.dma_start(
                            out[sb * 128:(sb + 1) * 128,
                                n * 512:(n + 1) * 512], st[:])

    nc.compile()
    return nc


def _host_shards(query, key, value, attn_mask, Wq, bq, Wk, bk, Wv, bv,
                 Wu, bu, Wf2, rel_table, causal, mm_dt=None):
    """Build the per-core input maps."""
    import ml_dtypes
    npdt = (np.dtype(ml_dtypes.bfloat16) if (mm_dt or MM_DT) == "bf16"
            else np.float32)
    _EYE128 = np.eye(128).astype(npdt)
    _ONES128 = np.ones((1, 128)).astype(npdt)
    in_maps = []
    # precompute per-head-group weight slices once (shared by 4 cores each)
    gdata = []
    for g in range(HGRP):
        c0, c1 = g * NHL * HD, (g + 1) * NHL * HD
        wq_c = np.ascontiguousarray(Wq[:, c0:c1]).astype(npdt)
        wk_c = np.ascontiguousarray(Wk[:, c0:c1]).astype(npdt)
        wv_c = np.ascontiguousarray(Wv[:, c0:c1]).astype(npdt)
        wu_c = np.ascontiguousarray(Wu[:, c0:c1]).astype(npdt)
        wf2_c = np.ascontiguousarray(Wf2[c0:c1, :]).astype(npdt)
        bq_c = np.ascontiguousarray(bq[c0:c1].reshape(NHL, 128).T)
        bk_c = np.ascontiguousarray(bk[c0:c1].reshape(NHL, 128).T)
        bu_c = np.ascontiguousarray(bu[c0:c1].reshape(NHL, 128).T)
        bv_c = np.ascontiguousarray(bv[c0:c1][None, :]).astype(npdt)
        # atab[h, r, y] = table[y - r, g*NHL + h]; for the causal variant the
        # table is pre-divided by SCALE and masked entries (m < MAXLEN-1,
        # i.e. key index > query index) are -1e5 so silu gives exactly 0.
        y = np.arange(2047)[None, :]
        r = np.arange(128)[:, None]
        idx = y - r                      # [128, 2047]
        valid = (idx >= 0) & (idx <= 2 * MAXLEN - 2)
        idxc = np.clip(idx, 0, 2 * MAXLEN - 2)
        cols = rel_table[:, g * NHL:(g + 1) * NHL]   # [2047, NHL]
        if causal:
            cols = cols / np.float32(SCALE)
            cols = np.where(np.arange(2047)[:, None] >= MAXLEN - 1, cols,
                            np.float32(-1e5))
            at = np.where(valid[:, :, None], cols[idxc], np.float32(-1e5))
            atab_c = np.ascontiguousarray(at.transpose(2, 0, 1)).astype(npdt)
        else:
            at = cols[idxc] * valid[:, :, None]
            atab_c = np.ascontiguousarray(
                at.transpose(2, 0, 1)).astype(np.float32)
        gdata.append((wq_c, wk_c, wv_c, wu_c, wf2_c, bq_c, bk_c, bu_c,
                      bv_c, atab_c))

    for c in range(NCORES):
        b, g = c // HGRP, c % HGRP
        (wq_c, wk_c, wv_c, wu_c, wf2_c, bq_c, bk_c, bu_c, bv_c,
         atab_c) = gdata[g]
        m = {
            "qT": np.ascontiguousarray(query[b].T).astype(npdt),
            "kT": np.ascontiguousarray(key[b].T).astype(npdt),
            "vT": np.ascontiguousarray(value[b].T).astype(npdt),
            "wq": wq_c, "wk": wk_c, "wv": wv_c, "wu": wu_c, "wf2": wf2_c,
            "bq": bq_c, "bk": bk_c, "bu": bu_c, "bv": bv_c, "atab": atab_c,
            "ones1": _ONES128,
        }
        mb = attn_mask[b]
        if causal:
            m["ident"] = _EYE128
        else:
            import ml_dtypes as _mld
            mf = np.empty((128, NHL, S), _mld.bfloat16)
            for jb in range(8):
                mf[:, jb, :] = mb[:, jb * 128:(jb + 1) * 128].T
            m["maskf"] = mf
        in_maps.append(m)
    return in_maps


def kernel(query, key, value, attn_mask, Wq, bq, Wk, bk, Wv, bv, Wu, bu,
           Wf2, bf2, rel_table):
    global LAST_EXEC_NS
    query = np.asarray(query, np.float32)
    key = np.asarray(key, np.float32)
    value = np.asarray(value, np.float32)
    attn_mask = np.asarray(attn_mask, bool)
    Wq, bq = np.asarray(Wq, np.float32), np.asarray(bq, np.float32)
    Wk, bk = np.asarray(Wk, np.float32), np.asarray(bk, np.float32)
    Wv, bv = np.asarray(Wv, np.float32), np.asarray(bv, np.float32)
    Wu, bu = np.asarray(Wu, np.float32), np.asarray(bu, np.float32)
    Wf2, bf2 = np.asarray(Wf2, np.float32), np.asarray(bf2, np.float32)
    rel_table = np.asarray(rel_table, np.float32)

    tril = np.tril(np.ones((S, S), bool))
    causal = all(np.array_equal(attn_mask[b], tril) for b in range(B))

    key_ = (causal, MM_DT)
    if key_ not in _CACHE:
        _CACHE[key_] = _build(causal)
    nc = _CACHE[key_]

    in_maps = _host_shards(query, key, value, attn_mask, Wq, bq, Wk, bk,
                           Wv, bv, Wu, bu, Wf2, rel_table, causal)
    res = run_bass_kernel_spmd(nc, in_maps, list(range(NCORES)), trace=TRACE)
    if res.exec_time_ns is not None:
        LAST_EXEC_NS = res.exec_time_ns

    outp = np.empty((B, S, H), np.float32)
    for b in range(B):
        outp[b] = (res.results[2 * b]["out"] + res.results[2 * b + 1]["out"]
                   + bf2[None, :])
    return outp


# revision 14
# speedup vs baseline: 1.5610x; 1.0026x over previous
"""Trainium2 Bass kernel for nn_BaselineModel_35175782154746 (dense transformer
block with SiLU attention + relative-position bias).

Sharding: 8 NeuronCores = 4 batches x 2 head-groups (8 heads each).
Each core computes, for its (batch b, head-group g):
    U, Q, K, V projections (columns g*1024:(g+1)*1024 of Wu/Wq/Wk/Wv),
    SiLU attention with rel-pos bias for its 8 heads,
    gated = out * U, partial = gated @ Wf2[g*1024:(g+1)*1024, :].
Host reduces: out[b] = partial[2b] + partial[2b+1] + bf2.

All matmuls run as float32r (full fp32 storage, fast PE mode) with N=512
moving dim. Activations/bias layouts are arranged so the contraction dim is
always on SBUF partitions (inputs are pre-transposed on host).
"""

import sys
import os

for _p in ("/root/.axon_site/_ro/trn_rl_repo", "/opt/trn_rl_repo"):
    if os.path.isdir(_p) and _p not in sys.path:
        sys.path.append(_p)

import numpy as np

import concourse.bass as bass
import concourse.mybir as mybir
import concourse.tile as tile
from concourse import bacc
from concourse.bass_utils import run_bass_kernel_spmd

B, S, H, NH, MAXLEN = 4, 1024, 2048, 16, 1024
HD = H // NH            # 128
NHL = 8                 # heads per core (local)
HGRP = 2                # head groups
NCORES = 8
KT16 = H // 128         # 16 k-tiles for the H contraction
SCALE = float(HD) ** -0.5

f32 = mybir.dt.float32
f32r = mybir.dt.float32r
bf16 = mybir.dt.bfloat16
SILU = mybir.ActivationFunctionType.Silu
MULT = mybir.AluOpType.mult
ADD = mybir.AluOpType.add

TRACE = False
LAST_EXEC_NS = None
MM_DT = "bf16"          # "bf16" or "f32r" matmul operand dtype
_CACHE = {}


def _build(causal: bool, mm_dt=None):
    mmdt = {"bf16": bf16, "f32r": f32r}[mm_dt or MM_DT]
    nc = bacc.Bacc("TRN2", target_bir_lowering=False, debug=False,
                   num_devices=NCORES)

    def din(name, shape, dt=f32):
        return nc.dram_tensor(name, shape, dt, kind="ExternalInput").ap()

    qT = din("qT", [H, S], mmdt)
    kT = din("kT", [H, S], mmdt)
    vT = din("vT", [H, S], mmdt)
    wq = din("wq", [H, NHL * HD], mmdt)
    wk = din("wk", [H, NHL * HD], mmdt)
    wv = din("wv", [H, NHL * HD], mmdt)
    wu = din("wu", [H, NHL * HD], mmdt)
    wf2 = din("wf2", [NHL * HD, H], mmdt)
    bq = din("bq", [128, NHL])
    bk = din("bk", [128, NHL])
    bu = din("bu", [128, NHL])
    bv = din("bv", [1, NHL * HD], mmdt)
    ones1 = din("ones1", [1, 128], mmdt)
    if causal:
        atab = din("atab", [NHL, 128, 2047], mmdt)
        ident = din("ident", [128, 128], mmdt)
    else:
        atab = din("atab", [NHL, 128, 2047])
        maskf = din("maskf", [128, NHL, S], bf16)
    out = nc.dram_tensor("out", [S, H], f32, kind="ExternalOutput").ap()

    with tile.TileContext(nc) as tc:
        with (
            tc.tile_pool(name="const", bufs=1) as constp,
            tc.tile_pool(name="gatedp", bufs=1) as gatedp,
        ):
            bq_t = constp.tile([128, NHL], f32, tag="bq")
            bk_t = constp.tile([128, NHL], f32, tag="bk")
            bu_t = constp.tile([128, NHL], f32, tag="bu")
            bv_t = constp.tile([1, NHL * HD], mmdt, tag="bv")
            ones_t = constp.tile([1, 128], mmdt, tag="ones1")
            if causal:
                id_t = constp.tile([128, 128], mmdt, tag="ident")

            gatedT = gatedp.tile([128, NHL, S], mmdt, tag="gatedT")
            w2_tiles = [gatedp.tile([128, NHL, 512], mmdt, tag=f"wf2_{n % 2}",
                                    name=f"wf2_{n}") for n in range(4)]

            with (
                tc.tile_pool(name="inres", bufs=1) as inres,
                tc.tile_pool(name="attres", bufs=1) as attres,
            ):
                qres = inres.tile([128, KT16, S], mmdt, tag="qres")
                kres = inres.tile([128, KT16, S], mmdt, tag="kres")
                vres = inres.tile([128, KT16, S], mmdt, tag="qres", name="vres")
                # split per-k loads so the first sweep starts after one chunk
                for k in range(KT16):
                    nc.sync.dma_start(qres[:, k, :], qT[k * 128:(k + 1) * 128, :])
                nc.sync.dma_start(bu_t[:], bu[:])
                nc.sync.dma_start(bq_t[:], bq[:])
                nc.sync.dma_start(bk_t[:], bk[:])
                nc.sync.dma_start(bv_t[:], bv[:])
                nc.sync.dma_start(ones_t[:], ones1[:])
                if causal:
                    nc.sync.dma_start(id_t[:], ident[:])
                for k in range(KT16):
                    nc.sync.dma_start(kres[:, k, :], kT[k * 128:(k + 1) * 128, :])
                for k in range(KT16):
                    nc.sync.dma_start(vres[:, k, :], vT[k * 128:(k + 1) * 128, :])

                UT = attres.tile([128, NHL, S], bf16, tag="UT")
                QT = attres.tile([128, NHL, S], mmdt, tag="QT")
                KTt = attres.tile([128, NHL, S], mmdt, tag="KT")
                V = attres.tile([128, NHL, S], mmdt, tag="V")
                at_tiles = [attres.tile([128, 2047], mmdt if causal else f32,
                                        tag=f"atab{h % (4 if causal else 2)}", name=f"atab{h}")
                            for h in range(NHL)]
                if not causal:
                    mask_t = attres.tile([128, NHL, S], bf16, tag="mask")
                    nc.sync.dma_start(mask_t[:], maskf[:])
                # prefetched during the projection phases (sync queue, after
                # the input loads)
                for h in range(NHL):
                    nc.sync.dma_start(at_tiles[h][:], atab[h])
                wf2r = wf2.rearrange("(cb p) n -> p cb n", p=128)
                for n in range(4):
                    nc.sync.dma_start(w2_tiles[n][:],
                                      wf2r[:, :, n * 512:(n + 1) * 512])

                # ---- projections U, Q, K (transposed outputs [HD, S]) ----
                with (
                    tc.tile_pool(name="win", bufs=6 if causal else 4) as winp,
                    tc.tile_pool(name="pps", bufs=1, space="PSUM") as ppsum,
                ):
                    for wdram, xres, btile, outtile in (
                        (wu, qres, bu_t, UT),
                        (wq, qres, bq_t, QT),
                        (wk, kres, bk_t, KTt),
                    ):
                        for ih in range(2):
                            ps = [ppsum.tile([128, 512], f32, tag=f"pp{h}",
                                             name=f"pp{h}")
                                  for h in range(NHL)]
                            for k in range(KT16):
                                wt = winp.tile([128, NHL * HD], mmdt, tag="win")
                                nc.gpsimd.dma_start(
                                    wt[:], wdram[k * 128:(k + 1) * 128, :])
                                for h in range(NHL):
                                    nc.tensor.matmul(
                                        ps[h][:],
                                        lhsT=wt[:, h * HD:(h + 1) * HD],
                                        rhs=qres[:, k, ih * 512:(ih + 1) * 512]
                                        if xres is qres
                                        else kres[:, k, ih * 512:(ih + 1) * 512],
                                        start=(k == 0), stop=(k == KT16 - 1))
                            for h in range(NHL):
                                nc.scalar.activation(
                                    outtile[:, h, ih * 512:(ih + 1) * 512],
                                    ps[h][:], SILU, bias=btile[:, h:h + 1])

                    # ---- projection V (natural layout [S, NHL*HD]) ----
                    for ch in range(2):
                        ps = [ppsum.tile([128, 512], f32, tag=f"pp{sb}",
                                         name=f"ppv{sb}")
                              for sb in range(8)]
                        for k in range(KT16):
                            wt = winp.tile([128, 512], mmdt, tag="wvin")
                            nc.gpsimd.dma_start(
                                wt[:], wv[k * 128:(k + 1) * 128,
                                          ch * 512:(ch + 1) * 512])
                            for sb in range(8):
                                nc.tensor.matmul(
                                    ps[sb][:],
                                    lhsT=vres[:, k, sb * 128:(sb + 1) * 128],
                                    rhs=wt[:],
                                    start=(k == 0), stop=False)
                        for sb in range(8):
                            nc.tensor.matmul(
                                ps[sb][:],
                                lhsT=ones_t[:],
                                rhs=bv_t[:, ch * 512:(ch + 1) * 512],
                                start=False, stop=True)
                            nc.scalar.activation(
                                V[:, sb, ch * 512:(ch + 1) * 512],
                                ps[sb][:], SILU)

                # ---- attention per head ----
                with (
                    tc.tile_pool(name="attnp", bufs=6) as attnp,
                    tc.tile_pool(name="pssc", bufs=6, space="PSUM") as pssc,
                    tc.tile_pool(name="psav", bufs=2, space="PSUM") as psav,
                ):
                    for h in range(NHL):
                        at = at_tiles[h]
                        for ih in range(2):
                            njb = (4 * ih + 4) if causal else 8
                            avp = psav.tile([128, 512], f32, tag="av")
                            chunks = [list(range(j, min(j + 3, njb)))
                                      for j in range(0, njb, 3)]
                            att_tiles = {}

                            def emit_scores(ch_, h=h, ih=ih, at=at,
                                            att_tiles=att_tiles):
                                for jb in ch_:
                                    scp = pssc.tile([128, 512], f32, tag="sc",
                                                    name=f"sc{h}_{ih}_{jb}")
                                    nc.tensor.matmul(
                                        scp[:],
                                        lhsT=KTt[:, h, jb * 128:(jb + 1) * 128],
                                        rhs=QT[:, h, ih * 512:(ih + 1) * 512],
                                        start=True, stop=not causal)
                                    att = attnp.tile([128, 512], mmdt,
                                                     tag="attn",
                                                     name=f"at{h}_{ih}_{jb}")
                                    d0 = ih * 512 - jb * 128 + MAXLEN - 1
                                    if causal:
                                        nc.tensor.matmul(
                                            scp[:], lhsT=id_t[:],
                                            rhs=at[:, d0:d0 + 512],
                                            start=False, stop=True)
                                        nc.scalar.activation(
                                            att[:], scp[:], SILU, scale=SCALE)
                                    else:
                                        nc.vector.scalar_tensor_tensor(
                                            att[:], scp[:], SCALE,
                                            at[:, d0:d0 + 512],
                                            op0=MULT, op1=ADD)
                                        nc.scalar.activation(att[:], att[:],
                                                             SILU)
                                        nc.vector.tensor_mul(
                                            att[:], att[:],
                                            mask_t[:, jb,
                                                   ih * 512:(ih + 1) * 512])
                                    att_tiles[jb] = att

                            emit_scores(chunks[0])
                            for ci, ch_ in enumerate(chunks):
                                if ci + 1 < len(chunks):
                                    emit_scores(chunks[ci + 1])
                                for jb in ch_:
                                    nc.tensor.matmul(
                                        avp[:],
                                        lhsT=V[:, jb, h * HD:(h + 1) * HD],
                                        rhs=att_tiles.pop(jb)[:],
                                        start=(jb == 0), stop=(jb == njb - 1))
                            nc.vector.tensor_mul(
                                gatedT[:, h, ih * 512:(ih + 1) * 512],
                                avp[:],
                                UT[:, h, ih * 512:(ih + 1) * 512])

            # ---- f2: partial = gatedT.T @ Wf2 (bf2 added on host) ----
            with (
                tc.tile_pool(name="stgp", bufs=3) as stgp,
                tc.tile_pool(name="psf2", bufs=4, space="PSUM") as psf2,
            ):
                for n in range(4):
                    w2 = w2_tiles[n]
                    for sb in range(8):
                        ps = psf2.tile([128, 512], f32, tag="f2")
                        for cb in range(NHL):
                            nc.tensor.matmul(
                                ps[:],
                                lhsT=gatedT[:, cb, sb * 128:(sb + 1) * 128],
                                rhs=w2[:, cb, :],
                                start=(cb == 0), stop=(cb == NHL - 1))
                        st = stgp.tile([128, 512], f32, tag="st")
                        nc.vector.tensor_copy(st[:], ps[:])
                        nc.sync.dma_start(
                            out[sb * 128:(sb + 1) * 128,
                                n * 512:(n + 1) * 512], st[:])

    nc.compile()
    return nc


def _host_shards(query, key, value, attn_mask, Wq, bq, Wk, bk, Wv, bv,
                 Wu, bu, Wf2, rel_table, causal, mm_dt=None):
    """Build the per-core input maps."""
    import ml_dtypes
    npdt = (np.dtype(ml_dtypes.bfloat16) if (mm_dt or MM_DT) == "bf16"
            else np.float32)
    _EYE128 = np.eye(128).astype(npdt)
    _ONES128 = np.ones((1, 128)).astype(npdt)
    in_maps = []
    # precompute per-head-group weight slices once (shared by 4 cores each)
    gdata = []
    for g in range(HGRP):
        c0, c1 = g * NHL * HD, (g + 1) * NHL * HD
        wq_c = np.ascontiguousarray(Wq[:, c0:c1]).astype(npdt)
        wk_c = np.ascontiguousarray(Wk[:, c0:c1]).astype(npdt)
        wv_c = np.ascontiguousarray(Wv[:, c0:c1]).astype(npdt)
        wu_c = np.ascontiguousarray(Wu[:, c0:c1]).astype(npdt)
        wf2_c = np.ascontiguousarray(Wf2[c0:c1, :]).astype(npdt)
        bq_c = np.ascontiguousarray(bq[c0:c1].reshape(NHL, 128).T)
        bk_c = np.ascontiguousarray(bk[c0:c1].reshape(NHL, 128).T)
        bu_c = np.ascontiguousarray(bu[c0:c1].reshape(NHL, 128).T)
        bv_c = np.ascontiguousarray(bv[c0:c1][None, :]).astype(npdt)
        # atab[h, r, y] = table[y - r, g*NHL + h]; for the causal variant the
        # table is pre-divided by SCALE and masked entries (m < MAXLEN-1,
        # i.e. key index > query index) are -1e5 so silu gives exactly 0.
        y = np.arange(2047)[None, :]
        r = np.arange(128)[:, None]
        idx = y - r                      # [128, 2047]
        valid = (idx >= 0) & (idx <= 2 * MAXLEN - 2)
        idxc = np.clip(idx, 0, 2 * MAXLEN - 2)
        cols = rel_table[:, g * NHL:(g + 1) * NHL]   # [2047, NHL]
        if causal:
            cols = cols / np.float32(SCALE)
            cols = np.where(np.arange(2047)[:, None] >= MAXLEN - 1, cols,
                            np.float32(-1e5))
            at = np.where(valid[:, :, None], cols[idxc], np.float32(-1e5))
            atab_c = np.ascontiguousarray(at.transpose(2, 0, 1)).astype(npdt)
        else:
            at = cols[idxc] * valid[:, :, None]
            atab_c = np.ascontiguousarray(
                at.transpose(2, 0, 1)).astype(np.float32)
        gdata.append((wq_c, wk_c, wv_c, wu_c, wf2_c, bq_c, bk_c, bu_c,
                      bv_c, atab_c))

    for c in range(NCORES):
        b, g = c // HGRP, c % HGRP
        (wq_c, wk_c, wv_c, wu_c, wf2_c, bq_c, bk_c, bu_c, bv_c,
         atab_c) = gdata[g]
        m = {
            "qT": np.ascontiguousarray(query[b].T).astype(npdt),
            "kT": np.ascontiguousarray(key[b].T).astype(npdt),
            "vT": np.ascontiguousarray(value[b].T).astype(npdt),
            "wq": wq_c, "wk": wk_c, "wv": wv_c, "wu": wu_c, "wf2": wf2_c,
            "bq": bq_c, "bk": bk_c, "bu": bu_c, "bv": bv_c, "atab": atab_c,
            "ones1": _ONES128,
        }
        mb = attn_mask[b]
        if causal:
            m["ident"] = _EYE128
        else:
            import ml_dtypes as _mld
            mf = np.empty((128, NHL, S), _mld.bfloat16)
            for jb in range(8):
                mf[:, jb, :] = mb[:, jb * 128:(jb + 1) * 128].T
            m["maskf"] = mf
        in_maps.append(m)
    return in_maps


def kernel(query, key, value, attn_mask, Wq, bq, Wk, bk, Wv, bv, Wu, bu,
           Wf2, bf2, rel_table):
    global LAST_EXEC_NS
    query = np.asarray(query, np.float32)
    key = np.asarray(key, np.float32)
    value = np.asarray(value, np.float32)
    attn_mask = np.asarray(attn_mask, bool)
    Wq, bq = np.asarray(Wq, np.float32), np.asarray(bq, np.float32)
    Wk, bk = np.asarray(Wk, np.float32), np.asarray(bk, np.float32)
    Wv, bv = np.asarray(Wv, np.float32), np.asarray(bv, np.float32)
    Wu, bu = np.asarray(Wu, np.float32), np.asarray(bu, np.float32)
    Wf2, bf2 = np.asarray(Wf2, np.float32), np.asarray(bf2, np.float32)
    rel_table = np.asarray(rel_table, np.float32)

    tril = np.tril(np.ones((S, S), bool))
    causal = all(np.array_equal(attn_mask[b], tril) for b in range(B))

    key_ = (causal, MM_DT)
    if key_ not in _CACHE:
        _CACHE[key_] = _build(causal)
    nc = _CACHE[key_]

    in_maps = _host_shards(query, key, value, attn_mask, Wq, bq, Wk, bk,
                           Wv, bv, Wu, bu, Wf2, rel_table, causal)
    res = run_bass_kernel_spmd(nc, in_maps, list(range(NCORES)), trace=TRACE)
    if res.exec_time_ns is not None:
        LAST_EXEC_NS = res.exec_time_ns

    outp = np.empty((B, S, H), np.float32)
    for b in range(B):
        outp[b] = (res.results[2 * b]["out"] + res.results[2 * b + 1]["out"]
                   + bf2[None, :])
    return outp


# revision 15
# speedup vs baseline: 1.5620x; 1.0007x over previous
"""Trainium2 Bass kernel for nn_BaselineModel_35175782154746 (dense transformer
block with SiLU attention + relative-position bias).

Sharding: 8 NeuronCores = 4 batches x 2 head-groups (8 heads each).
Each core computes, for its (batch b, head-group g):
    U, Q, K, V projections (columns g*1024:(g+1)*1024 of Wu/Wq/Wk/Wv),
    SiLU attention with rel-pos bias for its 8 heads,
    gated = out * U, partial = gated @ Wf2[g*1024:(g+1)*1024, :].
Host reduces: out[b] = partial[2b] + partial[2b+1] + bf2.

All matmuls run as float32r (full fp32 storage, fast PE mode) with N=512
moving dim. Activations/bias layouts are arranged so the contraction dim is
always on SBUF partitions (inputs are pre-transposed on host).
"""

import sys
import os

for _p in ("/root/.axon_site/_ro/trn_rl_repo", "/opt/trn_rl_repo"):
    if os.path.isdir(_p) and _p not in sys.path:
        sys.path.append(_p)

import numpy as np

import concourse.bass as bass
import concourse.mybir as mybir
import concourse.tile as tile
from concourse import bacc
from concourse.bass_utils import run_bass_kernel_spmd

B, S, H, NH, MAXLEN = 4, 1024, 2048, 16, 1024
HD = H // NH            # 128
NHL = 8                 # heads per core (local)
HGRP = 2                # head groups
NCORES = 8
KT16 = H // 128         # 16 k-tiles for the H contraction
SCALE = float(HD) ** -0.5

f32 = mybir.dt.float32
f32r = mybir.dt.float32r
bf16 = mybir.dt.bfloat16
SILU = mybir.ActivationFunctionType.Silu
MULT = mybir.AluOpType.mult
ADD = mybir.AluOpType.add

TRACE = False
LAST_EXEC_NS = None
MM_DT = "bf16"          # "bf16" or "f32r" matmul operand dtype
_CACHE = {}


def _build(causal: bool, mm_dt=None):
    mmdt = {"bf16": bf16, "f32r": f32r}[mm_dt or MM_DT]
    nc = bacc.Bacc("TRN2", target_bir_lowering=False, debug=False,
                   num_devices=NCORES)

    def din(name, shape, dt=f32):
        return nc.dram_tensor(name, shape, dt, kind="ExternalInput").ap()

    qT = din("qT", [H, S], mmdt)
    kT = din("kT", [H, S], mmdt)
    vT = din("vT", [H, S], mmdt)
    wq = din("wq", [H, NHL * HD], mmdt)
    wk = din("wk", [H, NHL * HD], mmdt)
    wv = din("wv", [H, NHL * HD], mmdt)
    wu = din("wu", [H, NHL * HD], mmdt)
    wf2 = din("wf2", [NHL * HD, H], mmdt)
    bq = din("bq", [128, NHL])
    bk = din("bk", [128, NHL])
    bu = din("bu", [128, NHL])
    bv = din("bv", [1, NHL * HD], mmdt)
    ones1 = din("ones1", [1, 128], mmdt)
    if causal:
        atab = din("atab", [NHL, 128, 2047], mmdt)
        ident = din("ident", [128, 128], mmdt)
    else:
        atab = din("atab", [NHL, 128, 2047])
        maskf = din("maskf", [128, NHL, S], bf16)
    out = nc.dram_tensor("out", [S, H], f32, kind="ExternalOutput").ap()

    with tile.TileContext(nc) as tc:
        with (
            tc.tile_pool(name="const", bufs=1) as constp,
            tc.tile_pool(name="gatedp", bufs=1) as gatedp,
        ):
            bq_t = constp.tile([128, NHL], f32, tag="bq")
            bk_t = constp.tile([128, NHL], f32, tag="bk")
            bu_t = constp.tile([128, NHL], f32, tag="bu")
            bv_t = constp.tile([1, NHL * HD], mmdt, tag="bv")
            ones_t = constp.tile([1, 128], mmdt, tag="ones1")
            if causal:
                id_t = constp.tile([128, 128], mmdt, tag="ident")

            gatedT = gatedp.tile([128, NHL, S], mmdt, tag="gatedT")
            w2_tiles = [gatedp.tile([128, NHL, 512], mmdt, tag=f"wf2_{n % 2}",
                                    name=f"wf2_{n}") for n in range(4)]

            with (
                tc.tile_pool(name="inres", bufs=1) as inres,
                tc.tile_pool(name="attres", bufs=1) as attres,
            ):
                qres = inres.tile([128, KT16, S], mmdt, tag="qres")
                kres = inres.tile([128, KT16, S], mmdt, tag="kres")
                vres = inres.tile([128, KT16, S], mmdt, tag="qres", name="vres")
                # split per-k loads so the first sweep starts after one chunk
                for k in range(KT16):
                    nc.sync.dma_start(qres[:, k, :], qT[k * 128:(k + 1) * 128, :])
                nc.sync.dma_start(bu_t[:], bu[:])
                nc.sync.dma_start(bq_t[:], bq[:])
                nc.sync.dma_start(bk_t[:], bk[:])
                nc.sync.dma_start(bv_t[:], bv[:])
                nc.sync.dma_start(ones_t[:], ones1[:])
                if causal:
                    nc.sync.dma_start(id_t[:], ident[:])
                for k in range(KT16):
                    nc.sync.dma_start(kres[:, k, :], kT[k * 128:(k + 1) * 128, :])
                for k in range(KT16):
                    nc.sync.dma_start(vres[:, k, :], vT[k * 128:(k + 1) * 128, :])

                UT = attres.tile([128, NHL, S], bf16, tag="UT")
                QT = attres.tile([128, NHL, S], mmdt, tag="QT")
                KTt = attres.tile([128, NHL, S], mmdt, tag="KT")
                V = attres.tile([128, NHL, S], mmdt, tag="V")
                at_tiles = [attres.tile([128, 2047], mmdt if causal else f32,
                                        tag=f"atab{h % (4 if causal else 2)}", name=f"atab{h}")
                            for h in range(NHL)]
                if not causal:
                    mask_t = attres.tile([128, NHL, S], bf16, tag="mask")
                    nc.sync.dma_start(mask_t[:], maskf[:])
                # prefetched during the projection phases (sync queue, after
                # the input loads)
                for h in range(NHL):
                    nc.sync.dma_start(at_tiles[h][:], atab[h])
                wf2r = wf2.rearrange("(cb p) n -> p cb n", p=128)
                for n in range(4):
                    nc.sync.dma_start(w2_tiles[n][:],
                                      wf2r[:, :, n * 512:(n + 1) * 512])

                # ---- projections U, Q, K (transposed outputs [HD, S]) ----
                with (
                    tc.tile_pool(name="win", bufs=6 if causal else 4) as winp,
                    tc.tile_pool(name="pps", bufs=1, space="PSUM") as ppsum,
                ):
                    for wdram, xres, btile, outtile in (
                        (wu, qres, bu_t, UT),
                        (wq, qres, bq_t, QT),
                        (wk, kres, bk_t, KTt),
                    ):
                        for ih in range(2):
                            ps = [ppsum.tile([128, 512], f32, tag=f"pp{h}",
                                             name=f"pp{h}")
                                  for h in range(NHL)]
                            for k in range(KT16):
                                wt = winp.tile([128, NHL * HD], mmdt, tag="win")
                                nc.gpsimd.dma_start(
                                    wt[:], wdram[k * 128:(k + 1) * 128, :])
                                for h in range(NHL):
                                    nc.tensor.matmul(
                                        ps[h][:],
                                        lhsT=wt[:, h * HD:(h + 1) * HD],
                                        rhs=qres[:, k, ih * 512:(ih + 1) * 512]
                                        if xres is qres
                                        else kres[:, k, ih * 512:(ih + 1) * 512],
                                        start=(k == 0), stop=(k == KT16 - 1))
                            for h in range(NHL):
                                nc.scalar.activation(
                                    outtile[:, h, ih * 512:(ih + 1) * 512],
                                    ps[h][:], SILU, bias=btile[:, h:h + 1])

                    # ---- projection V (natural layout [S, NHL*HD]) ----
                    for ch in range(2):
                        ps = [ppsum.tile([128, 512], f32, tag=f"pp{sb}",
                                         name=f"ppv{sb}")
                              for sb in range(8)]
                        for k in range(KT16):
                            wt = winp.tile([128, 512], mmdt, tag="wvin")
                            nc.gpsimd.dma_start(
                                wt[:], wv[k * 128:(k + 1) * 128,
                                          ch * 512:(ch + 1) * 512])
                            for sb in range(8):
                                nc.tensor.matmul(
                                    ps[sb][:],
                                    lhsT=vres[:, k, sb * 128:(sb + 1) * 128],
                                    rhs=wt[:],
                                    start=(k == 0), stop=False)
                        for sb in range(8):
                            nc.tensor.matmul(
                                ps[sb][:],
                                lhsT=ones_t[:],
                                rhs=bv_t[:, ch * 512:(ch + 1) * 512],
                                start=False, stop=True)
                            nc.scalar.activation(
                                V[:, sb, ch * 512:(ch + 1) * 512],
                                ps[sb][:], SILU)

                # ---- attention per head ----
                with (
                    tc.tile_pool(name="attnp", bufs=6) as attnp,
                    tc.tile_pool(name="psav", bufs=2, space="PSUM") as psav,
                    tc.tile_pool(name="pssc", bufs=6, space="PSUM") as pssc,
                ):
                    for h in range(NHL):
                        at = at_tiles[h]
                        for ih in range(2):
                            njb = (4 * ih + 4) if causal else 8
                            avp = psav.tile([128, 512], f32, tag="av")
                            chunks = [list(range(j, min(j + 3, njb)))
                                      for j in range(0, njb, 3)]
                            att_tiles = {}

                            def emit_scores(ch_, h=h, ih=ih, at=at,
                                            att_tiles=att_tiles):
                                for jb in ch_:
                                    scp = pssc.tile([128, 512], f32, tag="sc",
                                                    name=f"sc{h}_{ih}_{jb}")
                                    nc.tensor.matmul(
                                        scp[:],
                                        lhsT=KTt[:, h, jb * 128:(jb + 1) * 128],
                                        rhs=QT[:, h, ih * 512:(ih + 1) * 512],
                                        start=True, stop=not causal)
                                    att = attnp.tile([128, 512], mmdt,
                                                     tag="attn",
                                                     name=f"at{h}_{ih}_{jb}")
                                    d0 = ih * 512 - jb * 128 + MAXLEN - 1
                                    if causal:
                                        nc.tensor.matmul(
                                            scp[:], lhsT=id_t[:],
                                            rhs=at[:, d0:d0 + 512],
                                            start=False, stop=True)
                                        nc.scalar.activation(
                                            att[:], scp[:], SILU, scale=SCALE)
                                    else:
                                        nc.vector.scalar_tensor_tensor(
                                            att[:], scp[:], SCALE,
                                            at[:, d0:d0 + 512],
                                            op0=MULT, op1=ADD)
                                        nc.scalar.activation(att[:], att[:],
                                                             SILU)
                                        nc.vector.tensor_mul(
                                            att[:], att[:],
                                            mask_t[:, jb,
                                                   ih * 512:(ih + 1) * 512])
                                    att_tiles[jb] = att

                            emit_scores(chunks[0])
                            for ci, ch_ in enumerate(chunks):
                                if ci + 1 < len(chunks):
                                    emit_scores(chunks[ci + 1])
                                for jb in ch_:
                                    nc.tensor.matmul(
                                        avp[:],
                                        lhsT=V[:, jb, h * HD:(h + 1) * HD],
                                        rhs=att_tiles.pop(jb)[:],
                                        start=(jb == 0), stop=(jb == njb - 1))
                            nc.vector.tensor_mul(
                                gatedT[:, h, ih * 512:(ih + 1) * 512],
                                avp[:],
                                UT[:, h, ih * 512:(ih + 1) * 512])

            # ---- f2: partial = gatedT.T @ Wf2 (bf2 added on host) ----
            with (
                tc.tile_pool(name="stgp", bufs=3) as stgp,
                tc.tile_pool(name="psf2", bufs=4, space="PSUM") as psf2,
            ):
                for n in range(4):
                    w2 = w2_tiles[n]
                    for sb in range(8):
                        ps = psf2.tile([128, 512], f32, tag="f2")
                        for cb in range(NHL):
                            nc.tensor.matmul(
                                ps[:],
                                lhsT=gatedT[:, cb, sb * 128:(sb + 1) * 128],
                                rhs=w2[:, cb, :],
                                start=(cb == 0), stop=(cb == NHL - 1))
                        st = stgp.tile([128, 512], f32, tag="st")
                        nc.vector.tensor_copy(st[:], ps[:])
                        nc.sync.dma_start(
                            out[sb * 128:(sb + 1) * 128,
                                n * 512:(n + 1) * 512], st[:])

    nc.compile()
    return nc


def _host_shards(query, key, value, attn_mask, Wq, bq, Wk, bk, Wv, bv,
                 Wu, bu, Wf2, rel_table, causal, mm_dt=None):
    """Build the per-core input maps."""
    import ml_dtypes
    npdt = (np.dtype(ml_dtypes.bfloat16) if (mm_dt or MM_DT) == "bf16"
            else np.float32)
    _EYE128 = np.eye(128).astype(npdt)
    _ONES128 = np.ones((1, 128)).astype(npdt)
    in_maps = []
    # precompute per-head-group weight slices once (shared by 4 cores each)
    gdata = []
    for g in range(HGRP):
        c0, c1 = g * NHL * HD, (g + 1) * NHL * HD
        wq_c = np.ascontiguousarray(Wq[:, c0:c1]).astype(npdt)
        wk_c = np.ascontiguousarray(Wk[:, c0:c1]).astype(npdt)
        wv_c = np.ascontiguousarray(Wv[:, c0:c1]).astype(npdt)
        wu_c = np.ascontiguousarray(Wu[:, c0:c1]).astype(npdt)
        wf2_c = np.ascontiguousarray(Wf2[c0:c1, :]).astype(npdt)
        bq_c = np.ascontiguousarray(bq[c0:c1].reshape(NHL, 128).T)
        bk_c = np.ascontiguousarray(bk[c0:c1].reshape(NHL, 128).T)
        bu_c = np.ascontiguousarray(bu[c0:c1].reshape(NHL, 128).T)
        bv_c = np.ascontiguousarray(bv[c0:c1][None, :]).astype(npdt)
        # atab[h, r, y] = table[y - r, g*NHL + h]; for the causal variant the
        # table is pre-divided by SCALE and masked entries (m < MAXLEN-1,
        # i.e. key index > query index) are -1e5 so silu gives exactly 0.
        y = np.arange(2047)[None, :]
        r = np.arange(128)[:, None]
        idx = y - r                      # [128, 2047]
        valid = (idx >= 0) & (idx <= 2 * MAXLEN - 2)
        idxc = np.clip(idx, 0, 2 * MAXLEN - 2)
        cols = rel_table[:, g * NHL:(g + 1) * NHL]   # [2047, NHL]
        if causal:
            cols = cols / np.float32(SCALE)
            cols = np.where(np.arange(2047)[:, None] >= MAXLEN - 1, cols,
                            np.float32(-1e5))
            at = np.where(valid[:, :, None], cols[idxc], np.float32(-1e5))
            atab_c = np.ascontiguousarray(at.transpose(2, 0, 1)).astype(npdt)
        else:
            at = cols[idxc] * valid[:, :, None]
            atab_c = np.ascontiguousarray(
                at.transpose(2, 0, 1)).astype(np.float32)
        gdata.append((wq_c, wk_c, wv_c, wu_c, wf2_c, bq_c, bk_c, bu_c,
                      bv_c, atab_c))

    for c in range(NCORES):
        b, g = c // HGRP, c % HGRP
        (wq_c, wk_c, wv_c, wu_c, wf2_c, bq_c, bk_c, bu_c, bv_c,
         atab_c) = gdata[g]
        m = {
            "qT": np.ascontiguousarray(query[b].T).astype(npdt),
            "kT": np.ascontiguousarray(key[b].T).astype(npdt),
            "vT": np.ascontiguousarray(value[b].T).astype(npdt),
            "wq": wq_c, "wk": wk_c, "wv": wv_c, "wu": wu_c, "wf2": wf2_c,
            "bq": bq_c, "bk": bk_c, "bu": bu_c, "bv": bv_c, "atab": atab_c,
            "ones1": _ONES128,
        }
        mb = attn_mask[b]
        if causal:
            m["ident"] = _EYE128
        else:
            import ml_dtypes as _mld
            mf = np.empty((128, NHL, S), _mld.bfloat16)
            for jb in range(8):
                mf[:, jb, :] = mb[:, jb * 128:(jb + 1) * 128].T
            m["maskf"] = mf
        in_maps.append(m)
    return in_maps


def kernel(query, key, value, attn_mask, Wq, bq, Wk, bk, Wv, bv, Wu, bu,
           Wf2, bf2, rel_table):
    global LAST_EXEC_NS
    query = np.asarray(query, np.float32)
    key = np.asarray(key, np.float32)
    value = np.asarray(value, np.float32)
    attn_mask = np.asarray(attn_mask, bool)
    Wq, bq = np.asarray(Wq, np.float32), np.asarray(bq, np.float32)
    Wk, bk = np.asarray(Wk, np.float32), np.asarray(bk, np.float32)
    Wv, bv = np.asarray(Wv, np.float32), np.asarray(bv, np.float32)
    Wu, bu = np.asarray(Wu, np.float32), np.asarray(bu, np.float32)
    Wf2, bf2 = np.asarray(Wf2, np.float32), np.asarray(bf2, np.float32)
    rel_table = np.asarray(rel_table, np.float32)

    tril = np.tril(np.ones((S, S), bool))
    causal = all(np.array_equal(attn_mask[b], tril) for b in range(B))

    key_ = (causal, MM_DT)
    if key_ not in _CACHE:
        _CACHE[key_] = _build(causal)
    nc = _CACHE[key_]

    in_maps = _host_shards(query, key, value, attn_mask, Wq, bq, Wk, bk,
                           Wv, bv, Wu, bu, Wf2, rel_table, causal)
    res = run_bass_kernel_spmd(nc, in_maps, list(range(NCORES)), trace=TRACE)
    if res.exec_time_ns is not None:
        LAST_EXEC_NS = res.exec_time_ns

    outp = np.empty((B, S, H), np.float32)
    for b in range(B):
        outp[b] = (res.results[2 * b]["out"] + res.results[2 * b + 1]["out"]
                   + bf2[None, :])
    return outp


# revision 16
# speedup vs baseline: 1.5682x; 1.0040x over previous
"""Trainium2 Bass kernel for nn_BaselineModel_35175782154746 (dense transformer
block with SiLU attention + relative-position bias).

Sharding: 8 NeuronCores = 4 batches x 2 head-groups (8 heads each).
Each core computes, for its (batch b, head-group g):
    U, Q, K, V projections (columns g*1024:(g+1)*1024 of Wu/Wq/Wk/Wv),
    SiLU attention with rel-pos bias for its 8 heads,
    gated = out * U, partial = gated @ Wf2[g*1024:(g+1)*1024, :].
Host reduces: out[b] = partial[2b] + partial[2b+1] + bf2.

All matmuls run as float32r (full fp32 storage, fast PE mode) with N=512
moving dim. Activations/bias layouts are arranged so the contraction dim is
always on SBUF partitions (inputs are pre-transposed on host).
"""

import sys
import os

for _p in ("/root/.axon_site/_ro/trn_rl_repo", "/opt/trn_rl_repo"):
    if os.path.isdir(_p) and _p not in sys.path:
        sys.path.append(_p)

import numpy as np

import concourse.bass as bass
import concourse.mybir as mybir
import concourse.tile as tile
from concourse import bacc
from concourse.bass_utils import run_bass_kernel_spmd

B, S, H, NH, MAXLEN = 4, 1024, 2048, 16, 1024
HD = H // NH            # 128
NHL = 8                 # heads per core (local)
HGRP = 2                # head groups
NCORES = 8
KT16 = H // 128         # 16 k-tiles for the H contraction
SCALE = float(HD) ** -0.5

f32 = mybir.dt.float32
f32r = mybir.dt.float32r
bf16 = mybir.dt.bfloat16
SILU = mybir.ActivationFunctionType.Silu
MULT = mybir.AluOpType.mult
ADD = mybir.AluOpType.add

TRACE = False
LAST_EXEC_NS = None
MM_DT = "bf16"          # "bf16" or "f32r" matmul operand dtype
_CACHE = {}


def _build(causal: bool, mm_dt=None):
    mmdt = {"bf16": bf16, "f32r": f32r}[mm_dt or MM_DT]
    nc = bacc.Bacc("TRN2", target_bir_lowering=False, debug=False,
                   num_devices=NCORES)

    def din(name, shape, dt=f32):
        return nc.dram_tensor(name, shape, dt, kind="ExternalInput").ap()

    qT = din("qT", [H, S], mmdt)
    kT = din("kT", [H, S], mmdt)
    vT = din("vT", [H, S], mmdt)
    wq = din("wq", [H, NHL * HD], mmdt)
    wk = din("wk", [H, NHL * HD], mmdt)
    wv = din("wv", [H, NHL * HD], mmdt)
    wu = din("wu", [H, NHL * HD], mmdt)
    wf2 = din("wf2", [NHL * HD, H], mmdt)
    bq = din("bq", [128, NHL])
    bk = din("bk", [128, NHL])
    bu = din("bu", [128, NHL])
    bv = din("bv", [1, NHL * HD], mmdt)
    ones1 = din("ones1", [1, 128], mmdt)
    if causal:
        atab = din("atab", [NHL, 128, 2047], mmdt)
        ident = din("ident", [128, 128], mmdt)
    else:
        atab = din("atab", [NHL, 128, 2047])
        maskf = din("maskf", [128, NHL, S], bf16)
    out = nc.dram_tensor("out", [S, H], f32, kind="ExternalOutput").ap()

    with tile.TileContext(nc) as tc:
        with (
            tc.tile_pool(name="const", bufs=1) as constp,
            tc.tile_pool(name="gatedp", bufs=1) as gatedp,
        ):
            bq_t = constp.tile([128, NHL], f32, tag="bq")
            bk_t = constp.tile([128, NHL], f32, tag="bk")
            bu_t = constp.tile([128, NHL], f32, tag="bu")
            bv_t = constp.tile([1, NHL * HD], mmdt, tag="bv")
            ones_t = constp.tile([1, 128], mmdt, tag="ones1")
            if causal:
                id_t = constp.tile([128, 128], mmdt, tag="ident")

            gatedT = gatedp.tile([128, NHL, S], mmdt, tag="gatedT")
            w2_tiles = [gatedp.tile([128, NHL, 512], mmdt, tag=f"wf2_{n % 2}",
                                    name=f"wf2_{n}") for n in range(4)]

            with (
                tc.tile_pool(name="inres", bufs=1) as inres,
                tc.tile_pool(name="attres", bufs=1) as attres,
            ):
                qres = inres.tile([128, KT16, S], mmdt, tag="qres")
                kres = inres.tile([128, KT16, S], mmdt, tag="kres")
                vres = inres.tile([128, KT16, S], mmdt, tag="qres", name="vres")
                # split per-k loads so the first sweep starts after one chunk
                for k in range(KT16):
                    nc.sync.dma_start(vres[:, k, :], vT[k * 128:(k + 1) * 128, :])
                for k in range(KT16):
                    nc.sync.dma_start(qres[:, k, :], qT[k * 128:(k + 1) * 128, :])
                nc.sync.dma_start(bv_t[:], bv[:])
                nc.sync.dma_start(ones_t[:], ones1[:])
                nc.sync.dma_start(bu_t[:], bu[:])
                nc.sync.dma_start(bq_t[:], bq[:])
                nc.sync.dma_start(bk_t[:], bk[:])
                if causal:
                    nc.sync.dma_start(id_t[:], ident[:])
                for k in range(KT16):
                    nc.sync.dma_start(kres[:, k, :], kT[k * 128:(k + 1) * 128, :])

                UT = attres.tile([128, NHL, S], bf16, tag="UT")
                QT = attres.tile([128, NHL, S], mmdt, tag="QT")
                KTt = attres.tile([128, NHL, S], mmdt, tag="KT")
                V = attres.tile([128, NHL, S], mmdt, tag="V")
                at_tiles = [attres.tile([128, 2047], mmdt if causal else f32,
                                        tag=f"atab{h % (4 if causal else 2)}", name=f"atab{h}")
                            for h in range(NHL)]
                if not causal:
                    mask_t = attres.tile([128, NHL, S], bf16, tag="mask")
                    nc.sync.dma_start(mask_t[:], maskf[:])
                # prefetched during the projection phases (sync queue, after
                # the input loads)
                for h in range(NHL):
                    nc.sync.dma_start(at_tiles[h][:], atab[h])
                wf2r = wf2.rearrange("(cb p) n -> p cb n", p=128)
                for n in range(4):
                    nc.sync.dma_start(w2_tiles[n][:],
                                      wf2r[:, :, n * 512:(n + 1) * 512])

                # ---- projections U, Q, K (transposed outputs [HD, S]) ----
                with (
                    tc.tile_pool(name="win", bufs=6 if causal else 4) as winp,
                    tc.tile_pool(name="pps", bufs=1, space="PSUM") as ppsum,
                ):
                    # ---- projection V (natural layout [S, NHL*HD]) ----
                    for ch in range(2):
                        ps = [ppsum.tile([128, 512], f32, tag=f"pp{sb}",
                                         name=f"ppv{sb}")
                              for sb in range(8)]
                        for k in range(KT16):
                            wt = winp.tile([128, 512], mmdt, tag="wvin")
                            nc.gpsimd.dma_start(
                                wt[:], wv[k * 128:(k + 1) * 128,
                                          ch * 512:(ch + 1) * 512])
                            for sb in range(8):
                                nc.tensor.matmul(
                                    ps[sb][:],
                                    lhsT=vres[:, k, sb * 128:(sb + 1) * 128],
                                    rhs=wt[:],
                                    start=(k == 0), stop=False)
                        for sb in range(8):
                            nc.tensor.matmul(
                                ps[sb][:],
                                lhsT=ones_t[:],
                                rhs=bv_t[:, ch * 512:(ch + 1) * 512],
                                start=False, stop=True)
                            nc.scalar.activation(
                                V[:, sb, ch * 512:(ch + 1) * 512],
                                ps[sb][:], SILU)

                    for wdram, xres, btile, outtile in (
                        (wu, qres, bu_t, UT),
                        (wq, qres, bq_t, QT),
                        (wk, kres, bk_t, KTt),
                    ):
                        for ih in range(2):
                            ps = [ppsum.tile([128, 512], f32, tag=f"pp{h}",
                                             name=f"pp{h}")
                                  for h in range(NHL)]
                            for k in range(KT16):
                                wt = winp.tile([128, NHL * HD], mmdt, tag="win")
                                nc.gpsimd.dma_start(
                                    wt[:], wdram[k * 128:(k + 1) * 128, :])
                                for h in range(NHL):
                                    nc.tensor.matmul(
                                        ps[h][:],
                                        lhsT=wt[:, h * HD:(h + 1) * HD],
                                        rhs=qres[:, k, ih * 512:(ih + 1) * 512]
                                        if xres is qres
                                        else kres[:, k, ih * 512:(ih + 1) * 512],
                                        start=(k == 0), stop=(k == KT16 - 1))
                            for h in range(NHL):
                                nc.scalar.activation(
                                    outtile[:, h, ih * 512:(ih + 1) * 512],
                                    ps[h][:], SILU, bias=btile[:, h:h + 1])


                # ---- attention per head ----
                with (
                    tc.tile_pool(name="attnp", bufs=6) as attnp,
                    tc.tile_pool(name="psav", bufs=2, space="PSUM") as psav,
                    tc.tile_pool(name="pssc", bufs=6, space="PSUM") as pssc,
                ):
                    for h in range(NHL):
                        at = at_tiles[h]
                        for ih in range(2):
                            njb = (4 * ih + 4) if causal else 8
                            avp = psav.tile([128, 512], f32, tag="av")
                            chunks = [list(range(j, min(j + 3, njb)))
                                      for j in range(0, njb, 3)]
                            att_tiles = {}

                            def emit_scores(ch_, h=h, ih=ih, at=at,
                                            att_tiles=att_tiles):
                                for jb in ch_:
                                    scp = pssc.tile([128, 512], f32, tag="sc",
                                                    name=f"sc{h}_{ih}_{jb}")
                                    nc.tensor.matmul(
                                        scp[:],
                                        lhsT=KTt[:, h, jb * 128:(jb + 1) * 128],
                                        rhs=QT[:, h, ih * 512:(ih + 1) * 512],
                                        start=True, stop=not causal)
                                    att = attnp.tile([128, 512], mmdt,
                                                     tag="attn",
                                                     name=f"at{h}_{ih}_{jb}")
                                    d0 = ih * 512 - jb * 128 + MAXLEN - 1
                                    if causal:
                                        nc.tensor.matmul(
                                            scp[:], lhsT=id_t[:],
                                            rhs=at[:, d0:d0 + 512],
                                            start=False, stop=True)
                                        nc.scalar.activation(
                                            att[:], scp[:], SILU, scale=SCALE)
                                    else:
                                        nc.vector.scalar_tensor_tensor(
                                            att[:], scp[:], SCALE,
                                            at[:, d0:d0 + 512],
                                            op0=MULT, op1=ADD)
                                        nc.scalar.activation(att[:], att[:],
                                                             SILU)
                                        nc.vector.tensor_mul(
                                            att[:], att[:],
                                            mask_t[:, jb,
                                                   ih * 512:(ih + 1) * 512])
                                    att_tiles[jb] = att

                            emit_scores(chunks[0])
                            for ci, ch_ in enumerate(chunks):
                                if ci + 1 < len(chunks):
                                    emit_scores(chunks[ci + 1])
                                for jb in ch_:
                                    nc.tensor.matmul(
                                        avp[:],
                                        lhsT=V[:, jb, h * HD:(h + 1) * HD],
                                        rhs=att_tiles.pop(jb)[:],
                                        start=(jb == 0), stop=(jb == njb - 1))
                            nc.vector.tensor_mul(
                                gatedT[:, h, ih * 512:(ih + 1) * 512],
                                avp[:],
                                UT[:, h, ih * 512:(ih + 1) * 512])

            # ---- f2: partial = gatedT.T @ Wf2 (bf2 added on host) ----
            with (
                tc.tile_pool(name="stgp", bufs=3) as stgp,
                tc.tile_pool(name="psf2", bufs=4, space="PSUM") as psf2,
            ):
                for n in range(4):
                    w2 = w2_tiles[n]
                    for sb in range(8):
                        ps = psf2.tile([128, 512], f32, tag="f2")
                        for cb in range(NHL):
                            nc.tensor.matmul(
                                ps[:],
                                lhsT=gatedT[:, cb, sb * 128:(sb + 1) * 128],
                                rhs=w2[:, cb, :],
                                start=(cb == 0), stop=(cb == NHL - 1))
                        st = stgp.tile([128, 512], f32, tag="st")
                        nc.vector.tensor_copy(st[:], ps[:])
                        nc.sync.dma_start(
                            out[sb * 128:(sb + 1) * 128,
                                n * 512:(n + 1) * 512], st[:])

    nc.compile()
    return nc


def _host_shards(query, key, value, attn_mask, Wq, bq, Wk, bk, Wv, bv,
                 Wu, bu, Wf2, rel_table, causal, mm_dt=None):
    """Build the per-core input maps."""
    import ml_dtypes
    npdt = (np.dtype(ml_dtypes.bfloat16) if (mm_dt or MM_DT) == "bf16"
            else np.float32)
    _EYE128 = np.eye(128).astype(npdt)
    _ONES128 = np.ones((1, 128)).astype(npdt)
    in_maps = []
    # precompute per-head-group weight slices once (shared by 4 cores each)
    gdata = []
    for g in range(HGRP):
        c0, c1 = g * NHL * HD, (g + 1) * NHL * HD
        wq_c = np.ascontiguousarray(Wq[:, c0:c1]).astype(npdt)
        wk_c = np.ascontiguousarray(Wk[:, c0:c1]).astype(npdt)
        wv_c = np.ascontiguousarray(Wv[:, c0:c1]).astype(npdt)
        wu_c = np.ascontiguousarray(Wu[:, c0:c1]).astype(npdt)
        wf2_c = np.ascontiguousarray(Wf2[c0:c1, :]).astype(npdt)
        bq_c = np.ascontiguousarray(bq[c0:c1].reshape(NHL, 128).T)
        bk_c = np.ascontiguousarray(bk[c0:c1].reshape(NHL, 128).T)
        bu_c = np.ascontiguousarray(bu[c0:c1].reshape(NHL, 128).T)
        bv_c = np.ascontiguousarray(bv[c0:c1][None, :]).astype(npdt)
        # atab[h, r, y] = table[y - r, g*NHL + h]; for the causal variant the
        # table is pre-divided by SCALE and masked entries (m < MAXLEN-1,
        # i.e. key index > query index) are -1e5 so silu gives exactly 0.
        y = np.arange(2047)[None, :]
        r = np.arange(128)[:, None]
        idx = y - r                      # [128, 2047]
        valid = (idx >= 0) & (idx <= 2 * MAXLEN - 2)
        idxc = np.clip(idx, 0, 2 * MAXLEN - 2)
        cols = rel_table[:, g * NHL:(g + 1) * NHL]   # [2047, NHL]
        if causal:
            cols = cols / np.float32(SCALE)
            cols = np.where(np.arange(2047)[:, None] >= MAXLEN - 1, cols,
                            np.float32(-1e5))
            at = np.where(valid[:, :, None], cols[idxc], np.float32(-1e5))
            atab_c = np.ascontiguousarray(at.transpose(2, 0, 1)).astype(npdt)
        else:
            at = cols[idxc] * valid[:, :, None]
            atab_c = np.ascontiguousarray(
                at.transpose(2, 0, 1)).astype(np.float32)
        gdata.append((wq_c, wk_c, wv_c, wu_c, wf2_c, bq_c, bk_c, bu_c,
                      bv_c, atab_c))

    for c in range(NCORES):
        b, g = c // HGRP, c % HGRP
        (wq_c, wk_c, wv_c, wu_c, wf2_c, bq_c, bk_c, bu_c, bv_c,
         atab_c) = gdata[g]
        m = {
            "qT": np.ascontiguousarray(query[b].T).astype(npdt),
            "kT": np.ascontiguousarray(key[b].T).astype(npdt),
            "vT": np.ascontiguousarray(value[b].T).astype(npdt),
            "wq": wq_c, "wk": wk_c, "wv": wv_c, "wu": wu_c, "wf2": wf2_c,
            "bq": bq_c, "bk": bk_c, "bu": bu_c, "bv": bv_c, "atab": atab_c,
            "ones1": _ONES128,
        }
        mb = attn_mask[b]
        if causal:
            m["ident"] = _EYE128
        else:
            import ml_dtypes as _mld
            mf = np.empty((128, NHL, S), _mld.bfloat16)
            for jb in range(8):
                mf[:, jb, :] = mb[:, jb * 128:(jb + 1) * 128].T
            m["maskf"] = mf
        in_maps.append(m)
    return in_maps


def kernel(query, key, value, attn_mask, Wq, bq, Wk, bk, Wv, bv, Wu, bu,
           Wf2, bf2, rel_table):
    global LAST_EXEC_NS
    query = np.asarray(query, np.float32)
    key = np.asarray(key, np.float32)
    value = np.asarray(value, np.float32)
    attn_mask = np.asarray(attn_mask, bool)
    Wq, bq = np.asarray(Wq, np.float32), np.asarray(bq, np.float32)
    Wk, bk = np.asarray(Wk, np.float32), np.asarray(bk, np.float32)
    Wv, bv = np.asarray(Wv, np.float32), np.asarray(bv, np.float32)
    Wu, bu = np.asarray(Wu, np.float32), np.asarray(bu, np.float32)
    Wf2, bf2 = np.asarray(Wf2, np.float32), np.asarray(bf2, np.float32)
    rel_table = np.asarray(rel_table, np.float32)

    tril = np.tril(np.ones((S, S), bool))
    causal = all(np.array_equal(attn_mask[b], tril) for b in range(B))

    key_ = (causal, MM_DT)
    if key_ not in _CACHE:
        _CACHE[key_] = _build(causal)
    nc = _CACHE[key_]

    in_maps = _host_shards(query, key, value, attn_mask, Wq, bq, Wk, bk,
                           Wv, bv, Wu, bu, Wf2, rel_table, causal)
    res = run_bass_kernel_spmd(nc, in_maps, list(range(NCORES)), trace=TRACE)
    if res.exec_time_ns is not None:
        LAST_EXEC_NS = res.exec_time_ns

    outp = np.empty((B, S, H), np.float32)
    for b in range(B):
        outp[b] = (res.results[2 * b]["out"] + res.results[2 * b + 1]["out"]
                   + bf2[None, :])
    return outp


# revision 17
# speedup vs baseline: 1.5774x; 1.0059x over previous
"""Trainium2 Bass kernel for nn_BaselineModel_35175782154746 (dense transformer
block with SiLU attention + relative-position bias).

Sharding: 8 NeuronCores = 4 batches x 2 head-groups (8 heads each).
Each core computes, for its (batch b, head-group g):
    U, Q, K, V projections (columns g*1024:(g+1)*1024 of Wu/Wq/Wk/Wv),
    SiLU attention with rel-pos bias for its 8 heads,
    gated = out * U, partial = gated @ Wf2[g*1024:(g+1)*1024, :].
Host reduces: out[b] = partial[2b] + partial[2b+1] + bf2.

All matmuls run with bf16 operands (fp32 PSUM accumulation) at N=512 moving
dim — the TensorEngine's full-rate path. Layouts keep the contraction dim on
SBUF partitions (inputs pre-transposed on host). The rel-pos bias is added in
PSUM via an identity-matmul of a host-built shifted table (pre-divided by the
attention scale so ACT's native scale finishes scores = silu(scale*(QK+bias));
for the causal variant the mask is folded into that table as -1e5, which silu
maps to an exact 0.0 in fp32). A dense-mask fallback variant handles any
non-causal attn_mask exactly.
"""

import sys
import os

for _p in ("/root/.axon_site/_ro/trn_rl_repo", "/opt/trn_rl_repo"):
    if os.path.isdir(_p) and _p not in sys.path:
        sys.path.append(_p)

import numpy as np

import concourse.bass as bass
import concourse.mybir as mybir
import concourse.tile as tile
from concourse import bacc
from concourse.bass_utils import run_bass_kernel_spmd

B, S, H, NH, MAXLEN = 4, 1024, 2048, 16, 1024
HD = H // NH            # 128
NHL = 8                 # heads per core (local)
HGRP = 2                # head groups
NCORES = 8
KT16 = H // 128         # 16 k-tiles for the H contraction
SCALE = float(HD) ** -0.5

f32 = mybir.dt.float32
f32r = mybir.dt.float32r
bf16 = mybir.dt.bfloat16
SILU = mybir.ActivationFunctionType.Silu
MULT = mybir.AluOpType.mult
ADD = mybir.AluOpType.add

TRACE = False
LAST_EXEC_NS = None
MM_DT = "bf16"          # "bf16" or "f32r" matmul operand dtype
_CACHE = {}


def _build(causal: bool, mm_dt=None):
    mmdt = {"bf16": bf16, "f32r": f32r}[mm_dt or MM_DT]
    nc = bacc.Bacc("TRN2", target_bir_lowering=False, debug=False,
                   num_devices=NCORES)

    def din(name, shape, dt=f32):
        return nc.dram_tensor(name, shape, dt, kind="ExternalInput").ap()

    qT = din("qT", [H, S], mmdt)
    kT = din("kT", [H, S], mmdt)
    vT = din("vT", [H, S], mmdt)
    wq = din("wq", [H, NHL * HD], mmdt)
    wk = din("wk", [H, NHL * HD], mmdt)
    wv = din("wv", [H, NHL * HD], mmdt)
    wu = din("wu", [H, NHL * HD], mmdt)
    wf2 = din("wf2", [NHL * HD, H], mmdt)
    bq = din("bq", [128, NHL])
    bk = din("bk", [128, NHL])
    bu = din("bu", [128, NHL])
    bv = din("bv", [1, NHL * HD], mmdt)
    ones1 = din("ones1", [1, 128], mmdt)
    if causal:
        atab = din("atab", [NHL, 128, 2047], mmdt)
        ident = din("ident", [128, 128], mmdt)
    else:
        atab = din("atab", [NHL, 128, 2047])
        maskf = din("maskf", [128, NHL, S], bf16)
    out = nc.dram_tensor("out", [S, H], f32, kind="ExternalOutput").ap()

    with tile.TileContext(nc) as tc:
        with (
            tc.tile_pool(name="const", bufs=1) as constp,
            tc.tile_pool(name="gatedp", bufs=1) as gatedp,
        ):
            bq_t = constp.tile([128, NHL], f32, tag="bq")
            bk_t = constp.tile([128, NHL], f32, tag="bk")
            bu_t = constp.tile([128, NHL], f32, tag="bu")
            bv_t = constp.tile([1, NHL * HD], mmdt, tag="bv")
            ones_t = constp.tile([1, 128], mmdt, tag="ones1")
            if causal:
                id_t = constp.tile([128, 128], mmdt, tag="ident")

            gatedT = gatedp.tile([128, NHL, S], mmdt, tag="gatedT")
            w2_tiles = [gatedp.tile([128, NHL, 512], mmdt, tag=f"wf2_{n % 2}",
                                    name=f"wf2_{n}") for n in range(4)]

            with (
                tc.tile_pool(name="inres", bufs=1) as inres,
                tc.tile_pool(name="attres", bufs=1) as attres,
            ):
                qres = inres.tile([128, KT16, S], mmdt, tag="qres")
                kres = inres.tile([128, KT16, S], mmdt, tag="kres")
                vres = inres.tile([128, KT16, S], mmdt, tag="qres", name="vres")
                # split per-k loads so the first sweep starts after one chunk
                for k in range(KT16):
                    nc.sync.dma_start(vres[:, k, :], vT[k * 128:(k + 1) * 128, :])
                for k in range(KT16):
                    nc.sync.dma_start(qres[:, k, :], qT[k * 128:(k + 1) * 128, :])
                nc.sync.dma_start(bv_t[:], bv[:])
                nc.sync.dma_start(ones_t[:], ones1[:])
                nc.sync.dma_start(bu_t[:], bu[:])
                nc.sync.dma_start(bq_t[:], bq[:])
                nc.sync.dma_start(bk_t[:], bk[:])
                if causal:
                    nc.sync.dma_start(id_t[:], ident[:])
                for k in range(KT16):
                    nc.sync.dma_start(kres[:, k, :], kT[k * 128:(k + 1) * 128, :])

                UT = attres.tile([128, NHL, S], bf16, tag="UT")
                QT = attres.tile([128, NHL, S], mmdt, tag="QT")
                KTt = attres.tile([128, NHL, S], mmdt, tag="KT")
                V = attres.tile([128, NHL, S], mmdt, tag="V")
                at_tiles = [attres.tile([128, 2047], mmdt if causal else f32,
                                        tag=f"atab{h % (4 if causal else 2)}", name=f"atab{h}")
                            for h in range(NHL)]
                if not causal:
                    mask_t = attres.tile([128, NHL, S], bf16, tag="mask")
                    nc.sync.dma_start(mask_t[:], maskf[:])
                # prefetched during the projection phases (sync queue, after
                # the input loads)
                for h in range(NHL):
                    nc.sync.dma_start(at_tiles[h][:], atab[h])
                wf2r = wf2.rearrange("(cb p) n -> p cb n", p=128)
                for n in range(4):
                    nc.sync.dma_start(w2_tiles[n][:],
                                      wf2r[:, :, n * 512:(n + 1) * 512])

                # ---- projections U, Q, K (transposed outputs [HD, S]) ----
                with (
                    tc.tile_pool(name="win", bufs=6 if causal else 4) as winp,
                    tc.tile_pool(name="pps", bufs=1, space="PSUM") as ppsum,
                ):
                    # ---- projection V (natural layout [S, NHL*HD]) ----
                    for ch in range(2):
                        ps = [ppsum.tile([128, 512], f32, tag=f"pp{sb}",
                                         name=f"ppv{sb}")
                              for sb in range(8)]
                        for k in range(KT16):
                            wt = winp.tile([128, 512], mmdt, tag="wvin")
                            nc.gpsimd.dma_start(
                                wt[:], wv[k * 128:(k + 1) * 128,
                                          ch * 512:(ch + 1) * 512])
                            for sb in range(8):
                                nc.tensor.matmul(
                                    ps[sb][:],
                                    lhsT=vres[:, k, sb * 128:(sb + 1) * 128],
                                    rhs=wt[:],
                                    start=(k == 0), stop=False)
                        for sb in range(8):
                            nc.tensor.matmul(
                                ps[sb][:],
                                lhsT=ones_t[:],
                                rhs=bv_t[:, ch * 512:(ch + 1) * 512],
                                start=False, stop=True)
                            nc.scalar.activation(
                                V[:, sb, ch * 512:(ch + 1) * 512],
                                ps[sb][:], SILU)

                    for wdram, xres, btile, outtile in (
                        (wu, qres, bu_t, UT),
                        (wq, qres, bq_t, QT),
                        (wk, kres, bk_t, KTt),
                    ):
                        for ih in range(2):
                            ps = [ppsum.tile([128, 512], f32, tag=f"pp{h}",
                                             name=f"pp{h}")
                                  for h in range(NHL)]
                            for k in range(KT16):
                                wt = winp.tile([128, NHL * HD], mmdt, tag="win")
                                nc.gpsimd.dma_start(
                                    wt[:], wdram[k * 128:(k + 1) * 128, :])
                                for h in range(NHL):
                                    nc.tensor.matmul(
                                        ps[h][:],
                                        lhsT=wt[:, h * HD:(h + 1) * HD],
                                        rhs=qres[:, k, ih * 512:(ih + 1) * 512]
                                        if xres is qres
                                        else kres[:, k, ih * 512:(ih + 1) * 512],
                                        start=(k == 0), stop=(k == KT16 - 1))
                            for h in range(NHL):
                                nc.scalar.activation(
                                    outtile[:, h, ih * 512:(ih + 1) * 512],
                                    ps[h][:], SILU, bias=btile[:, h:h + 1])


                # ---- attention per head ----
                with (
                    tc.tile_pool(name="attnp", bufs=6) as attnp,
                    tc.tile_pool(name="psav", bufs=2, space="PSUM") as psav,
                    tc.tile_pool(name="pssc", bufs=6, space="PSUM") as pssc,
                ):
                    for h in range(NHL):
                        at = at_tiles[h]
                        for ih in range(2):
                            njb = (4 * ih + 4) if causal else 8
                            avp = psav.tile([128, 512], f32, tag="av")
                            chunks = [list(range(j, min(j + 3, njb)))
                                      for j in range(0, njb, 3)]
                            att_tiles = {}

                            def emit_scores(ch_, h=h, ih=ih, at=at,
                                            att_tiles=att_tiles):
                                for jb in ch_:
                                    scp = pssc.tile([128, 512], f32, tag="sc",
                                                    name=f"sc{h}_{ih}_{jb}")
                                    nc.tensor.matmul(
                                        scp[:],
                                        lhsT=KTt[:, h, jb * 128:(jb + 1) * 128],
                                        rhs=QT[:, h, ih * 512:(ih + 1) * 512],
                                        start=True, stop=not causal)
                                    att = attnp.tile([128, 512], mmdt,
                                                     tag="attn",
                                                     name=f"at{h}_{ih}_{jb}")
                                    d0 = ih * 512 - jb * 128 + MAXLEN - 1
                                    if causal:
                                        nc.tensor.matmul(
                                            scp[:], lhsT=id_t[:],
                                            rhs=at[:, d0:d0 + 512],
                                            start=False, stop=True)
                                        nc.scalar.activation(
                                            att[:], scp[:], SILU, scale=SCALE)
                                    else:
                                        nc.vector.scalar_tensor_tensor(
                                            att[:], scp[:], SCALE,
                                            at[:, d0:d0 + 512],
                                            op0=MULT, op1=ADD)
                                        nc.scalar.activation(att[:], att[:],
                                                             SILU)
                                        nc.vector.tensor_mul(
                                            att[:], att[:],
                                            mask_t[:, jb,
                                                   ih * 512:(ih + 1) * 512])
                                    att_tiles[jb] = att

                            emit_scores(chunks[0])
                            for ci, ch_ in enumerate(chunks):
                                if ci + 1 < len(chunks):
                                    emit_scores(chunks[ci + 1])
                                for jb in ch_:
                                    nc.tensor.matmul(
                                        avp[:],
                                        lhsT=V[:, jb, h * HD:(h + 1) * HD],
                                        rhs=att_tiles.pop(jb)[:],
                                        start=(jb == 0), stop=(jb == njb - 1))
                            nc.vector.tensor_mul(
                                gatedT[:, h, ih * 512:(ih + 1) * 512],
                                avp[:],
                                UT[:, h, ih * 512:(ih + 1) * 512])

            # ---- f2: partial = gatedT.T @ Wf2 (bf2 added on host) ----
            with (
                tc.tile_pool(name="stgp", bufs=3) as stgp,
                tc.tile_pool(name="psf2", bufs=4, space="PSUM") as psf2,
            ):
                for n in range(4):
                    w2 = w2_tiles[n]
                    for sb in range(8):
                        ps = psf2.tile([128, 512], f32, tag="f2")
                        for cb in range(NHL):
                            nc.tensor.matmul(
                                ps[:],
                                lhsT=gatedT[:, cb, sb * 128:(sb + 1) * 128],
                                rhs=w2[:, cb, :],
                                start=(cb == 0), stop=(cb == NHL - 1))
                        st = stgp.tile([128, 512], f32, tag="st")
                        nc.vector.tensor_copy(st[:], ps[:])
                        nc.sync.dma_start(
                            out[sb * 128:(sb + 1) * 128,
                                n * 512:(n + 1) * 512], st[:])

    nc.compile()
    return nc


def _host_shards(query, key, value, attn_mask, Wq, bq, Wk, bk, Wv, bv,
                 Wu, bu, Wf2, rel_table, causal, mm_dt=None):
    """Build the per-core input maps."""
    import ml_dtypes
    npdt = (np.dtype(ml_dtypes.bfloat16) if (mm_dt or MM_DT) == "bf16"
            else np.float32)
    _EYE128 = np.eye(128).astype(npdt)
    _ONES128 = np.ones((1, 128)).astype(npdt)
    in_maps = []
    # precompute per-head-group weight slices once (shared by 4 cores each)
    gdata = []
    for g in range(HGRP):
        c0, c1 = g * NHL * HD, (g + 1) * NHL * HD
        wq_c = np.ascontiguousarray(Wq[:, c0:c1]).astype(npdt)
        wk_c = np.ascontiguousarray(Wk[:, c0:c1]).astype(npdt)
        wv_c = np.ascontiguousarray(Wv[:, c0:c1]).astype(npdt)
        wu_c = np.ascontiguousarray(Wu[:, c0:c1]).astype(npdt)
        wf2_c = np.ascontiguousarray(Wf2[c0:c1, :]).astype(npdt)
        bq_c = np.ascontiguousarray(bq[c0:c1].reshape(NHL, 128).T)
        bk_c = np.ascontiguousarray(bk[c0:c1].reshape(NHL, 128).T)
        bu_c = np.ascontiguousarray(bu[c0:c1].reshape(NHL, 128).T)
        bv_c = np.ascontiguousarray(bv[c0:c1][None, :]).astype(npdt)
        # atab[h, r, y] = table[y - r, g*NHL + h]; for the causal variant the
        # table is pre-divided by SCALE and masked entries (m < MAXLEN-1,
        # i.e. key index > query index) are -1e5 so silu gives exactly 0.
        y = np.arange(2047)[None, :]
        r = np.arange(128)[:, None]
        idx = y - r                      # [128, 2047]
        valid = (idx >= 0) & (idx <= 2 * MAXLEN - 2)
        idxc = np.clip(idx, 0, 2 * MAXLEN - 2)
        cols = rel_table[:, g * NHL:(g + 1) * NHL]   # [2047, NHL]
        if causal:
            cols = cols / np.float32(SCALE)
            cols = np.where(np.arange(2047)[:, None] >= MAXLEN - 1, cols,
                            np.float32(-1e5))
            at = np.where(valid[:, :, None], cols[idxc], np.float32(-1e5))
            atab_c = np.ascontiguousarray(at.transpose(2, 0, 1)).astype(npdt)
        else:
            at = cols[idxc] * valid[:, :, None]
            atab_c = np.ascontiguousarray(
                at.transpose(2, 0, 1)).astype(np.float32)
        gdata.append((wq_c, wk_c, wv_c, wu_c, wf2_c, bq_c, bk_c, bu_c,
                      bv_c, atab_c))

    for c in range(NCORES):
        b, g = c // HGRP, c % HGRP
        (wq_c, wk_c, wv_c, wu_c, wf2_c, bq_c, bk_c, bu_c, bv_c,
         atab_c) = gdata[g]
        m = {
            "qT": np.ascontiguousarray(query[b].T).astype(npdt),
            "kT": np.ascontiguousarray(key[b].T).astype(npdt),
            "vT": np.ascontiguousarray(value[b].T).astype(npdt),
            "wq": wq_c, "wk": wk_c, "wv": wv_c, "wu": wu_c, "wf2": wf2_c,
            "bq": bq_c, "bk": bk_c, "bu": bu_c, "bv": bv_c, "atab": atab_c,
            "ones1": _ONES128,
        }
        mb = attn_mask[b]
        if causal:
            m["ident"] = _EYE128
        else:
            import ml_dtypes as _mld
            mf = np.empty((128, NHL, S), _mld.bfloat16)
            for jb in range(8):
                mf[:, jb, :] = mb[:, jb * 128:(jb + 1) * 128].T
            m["maskf"] = mf
        in_maps.append(m)
    return in_maps


def kernel(query, key, value, attn_mask, Wq, bq, Wk, bk, Wv, bv, Wu, bu,
           Wf2, bf2, rel_table):
    global LAST_EXEC_NS
    query = np.asarray(query, np.float32)
    key = np.asarray(key, np.float32)
    value = np.asarray(value, np.float32)
    attn_mask = np.asarray(attn_mask, bool)
    Wq, bq = np.asarray(Wq, np.float32), np.asarray(bq, np.float32)
    Wk, bk = np.asarray(Wk, np.float32), np.asarray(bk, np.float32)
    Wv, bv = np.asarray(Wv, np.float32), np.asarray(bv, np.float32)
    Wu, bu = np.asarray(Wu, np.float32), np.asarray(bu, np.float32)
    Wf2, bf2 = np.asarray(Wf2, np.float32), np.asarray(bf2, np.float32)
    rel_table = np.asarray(rel_table, np.float32)

    tril = np.tril(np.ones((S, S), bool))
    causal = all(np.array_equal(attn_mask[b], tril) for b in range(B))

    key_ = (causal, MM_DT)
    if key_ not in _CACHE:
        _CACHE[key_] = _build(causal)
    nc = _CACHE[key_]

    in_maps = _host_shards(query, key, value, attn_mask, Wq, bq, Wk, bk,
                           Wv, bv, Wu, bu, Wf2, rel_table, causal)
    res = run_bass_kernel_spmd(nc, in_maps, list(range(NCORES)), trace=TRACE)
    if res.exec_time_ns is not None:
        LAST_EXEC_NS = res.exec_time_ns

    outp = np.empty((B, S, H), np.float32)
    for b in range(B):
        outp[b] = (res.results[2 * b]["out"] + res.results[2 * b + 1]["out"]
                   + bf2[None, :])
    return outp
